# revision 13
# baseline (speedup 1.0000x reference)
"""DissipativeThetaRINN Trainium2 (Bass/Tile) kernel — 8-core data parallel.

Strategy (pure data parallel, per sharding hint):
  - Batch B=2048 is split across 8 NeuronCores (256 rows/core); the tiny
    controller matrices and value-MLP weights are replicated.
  - On-device layout is transposed: features on SBUF partitions, batch on
    the free dimension (256 columns per core).
  - The implicit layer w = tanh(Cv x + Dvy y + Dvw w) is solved with only
    N_ITERS=3 tanh evaluations: iteration 0 uses a linear-solve warm start
    w0 = tanh(M c) with M = (I - g Dvw)^-1, g=0.8, folded host-side into
    the const matmul (zero extra device work).  Each remaining iteration is
    one 256-wide matmul pair into PSUM + one 256-wide tanh.  All matmuls
    span the full 256-column batch — on TRN2 the per-instruction overhead
    (LDWEIGHTS ~100ns + drain ~170ns) dominates 128-col streams, so fewer,
    wider instructions beat a 2-chunk ping-pong.
  - x_next (forward Euler) uses the second-to-last w iterate so the final
    tanh is off the timestep-boundary critical path (error ~1e-5, verified
    in fp16-faithful numpy sim: total rel_l2 ≈ 5.5e-3 vs 2e-2 budget).
  - Matmuls run in fp16 (PSUM accumulates fp32); DT is pre-folded into the
    recurrence weights; the x accumulator stays fp32 on device.
  - The value MLP is computed in groups of 4 timesteps, packed 2-per-128
    partitions with block-diagonal weights, scheduled into engine gaps.
  - log_stds broadcast and the +b2 value bias are applied host-side.
"""
import numpy as np
import concourse.bass as bass
import concourse.mybir as mybir
import concourse.tile as tile
from concourse import bacc
from concourse.bass_utils import run_bass_kernel_spmd

dt = mybir.dt
AF = mybir.ActivationFunctionType

# problem shape (hardcoded per contract)
BFULL, TFULL = 2048, 128
S, NL, IN, OUT, H = 16, 128, 32, 8, 64
DT = 0.01
N_CORES = 8
N_ITERS = 3    # tanh evaluations per timestep (incl. warm-start iteration)
G_INIT = 0.8   # warm-start gain: w0 = tanh((I - g Dvw)^-T c)
VG = 4         # value-MLP timestep group (packed 2x2 onto 128 partitions)


def build_kernel(T=TFULL, B=BFULL // N_CORES, n_iters=N_ITERS):
    nc = bacc.Bacc(None, target_bir_lowering=False)
    f32, f16 = dt.float32, dt.float16
    assert n_iters >= 2

    obsT16 = nc.dram_tensor("obsT16", [T, IN, B], f16, kind="ExternalInput")
    x0T = nc.dram_tensor("x0T", [S, B], f32, kind="ExternalInput")
    Wdvw = nc.dram_tensor("Wdvw", [NL, NL], f16, kind="ExternalInput")
    Wcd = nc.dram_tensor("Wcd", [S + IN, NL], f16, kind="ExternalInput")
    Wcd0 = nc.dram_tensor("Wcd0", [S + IN, NL], f16, kind="ExternalInput")
    Wxu = nc.dram_tensor("Wxu", [S + IN, 32 + OUT], f16, kind="ExternalInput")
    Wuw = nc.dram_tensor("Wuw", [NL, OUT], f16, kind="ExternalInput")
    Wxw = nc.dram_tensor("Wxw", [NL, S], f16, kind="ExternalInput")
    Wv0b = nc.dram_tensor("Wv0b", [2 * IN, 2 * H], f16, kind="ExternalInput")
    Wv1b = nc.dram_tensor("Wv1b", [2 * H, 2 * H], f16, kind="ExternalInput")
    Wv2b = nc.dram_tensor("Wv2b", [2 * H, 2], f16, kind="ExternalInput")
    b0v = nc.dram_tensor("b0v", [2 * H, 1], f32, kind="ExternalInput")
    b1v = nc.dram_tensor("b1v", [2 * H, 1], f32, kind="ExternalInput")

    u_out = nc.dram_tensor("u_out", [T, OUT, B], f32, kind="ExternalOutput")
    v_out = nc.dram_tensor("v_out", [T, B], f32, kind="ExternalOutput")

    B2 = 2 * B
    n_groups = (T + VG - 1) // VG

    with tile.TileContext(nc) as tc:
        with tc.tile_pool(name="wts", bufs=1) as wts, \
             tc.tile_pool(name="xyp", bufs=3) as xyp, \
             tc.tile_pool(name="xtp", bufs=2) as xtp, \
             tc.tile_pool(name="wp", bufs=3) as wp, \
             tc.tile_pool(name="obp", bufs=2) as obp, \
             tc.tile_pool(name="vp", bufs=2) as vp, \
             tc.tile_pool(name="up", bufs=2) as up, \
             tc.tile_pool(name="pw", bufs=3, space="PSUM") as pwp, \
             tc.tile_pool(name="pxu", bufs=2, space="PSUM") as pxup, \
             tc.tile_pool(name="ph", bufs=1, space="PSUM") as php, \
             tc.tile_pool(name="pv", bufs=1, space="PSUM") as pvp:

            def wt(name, dram, shape, dtp):
                tl = wts.tile(shape, dtp, name=name)
                nc.sync.dma_start(tl[:], dram[:])
                return tl
            wdvw = wt("wdvw", Wdvw, [NL, NL], f16)
            wcd = wt("wcd", Wcd, [S + IN, NL], f16)
            wcd0 = wt("wcd0", Wcd0, [S + IN, NL], f16)
            wxu = wt("wxu", Wxu, [S + IN, 32 + OUT], f16)
            wuw = wt("wuw", Wuw, [NL, OUT], f16)
            wxw = wt("wxw", Wxw, [NL, S], f16)
            wv0b = wt("wv0b", Wv0b, [2 * IN, 2 * H], f16)
            wv1b = wt("wv1b", Wv1b, [2 * H, 2 * H], f16)
            wv2b = wt("wv2b", Wv2b, [2 * H, 2], f16)
            b0 = wt("b0", b0v, [2 * H, 1], f32)
            b1 = wt("b1", b1v, [2 * H, 1], f32)

            def load_obs4(g):
                """obs4 [64, 2B]: rows 0:32 = ts {4g, 4g+1}, rows 32:64 =
                ts {4g+2, 4g+3} (two col blocks of B each)."""
                t0 = g * VG
                ob = obp.tile([2 * IN, B2], f16, name=f"obs4_{g}", tag="obs4")
                nc.sync.dma_start(
                    ob[0:IN, :].rearrange("k (g1 b) -> k g1 b", g1=2),
                    obsT16[t0:t0 + 2].transpose([1, 0, 2]))
                nc.sync.dma_start(
                    ob[IN:, :].rearrange("k (g1 b) -> k g1 b", g1=2),
                    obsT16[t0 + 2:t0 + 4].transpose([1, 0, 2]))
                return ob

            obs4 = load_obs4(0)
            obs4_next = load_obs4(1) if n_groups > 1 else None

            # xy_h [48, B] f16: rows 0:32 = y^T, rows 32:48 = x^T
            xt_r = xtp.tile([S, B], f32, name="xt_r0", tag="xt_r")
            nc.sync.dma_start(xt_r[:], x0T[:])
            xy_h = xyp.tile([S + IN, B], f16, name="xy_h0", tag="xy_h")
            nc.sync.dma_start(xy_h[0:IN, :], obsT16[0])
            nc.vector.tensor_copy(xy_h[IN:, :], xt_r[:])

            u4 = None
            u_pend = None  # (pu_tile, w_final, t): wuw matmul deferred to t+1
            u_last = None

            def copy_u(tp):
                nonlocal u4
                g4p = tp % VG
                if g4p == 0:
                    u4 = up.tile([OUT, VG * B], f32, name=f"u4_{tp // VG}", tag="u4")
                nc.vector.tensor_copy(u4[:, g4p * B:(g4p + 1) * B], u_last)
                if g4p == VG - 1:
                    nc.sync.dma_start(
                        u_out[tp - VG + 1:tp + 1].transpose([1, 0, 2]),
                        u4[:].rearrange("o (g1 b) -> o g1 b", g1=VG))

            for t in range(T):
                g, g4 = t // VG, t % VG
                # -- PE front: warm-start + it1-const (need only xy), value
                #    layer-0, then t-1's deferred u close.
                p0 = pwp.tile([NL, B], f32, name=f"pw{t}_0", tag="pw")
                nc.tensor.matmul(p0[:], wcd0[:], xy_h[:], start=True, stop=True)
                p1 = pwp.tile([NL, B], f32, name=f"pw{t}_1", tag="pw")
                nc.tensor.matmul(p1[:], wcd[:], xy_h[:], start=True, stop=False)
                if g4 == 0:
                    ph = php.tile([2 * H, B2], f32, name=f"ph_{g}", tag="ph")
                    nc.tensor.matmul(ph[:], wv0b[:], obs4[:], start=True, stop=True)
                if u_pend is not None:
                    pu_p, w_p, _ = u_pend
                    nc.tensor.matmul(pu_p, wuw[:], w_p[:], start=False, stop=True,
                                     skip_group_check=True)
                    u_last, u_pend = pu_p, None

                w0 = wp.tile([NL, B], f16, name=f"w{t}_0", tag="w")
                nc.scalar.activation(w0[:], p0[:], AF.Tanh)           # tanh0
                if g4 == 0:
                    h1 = vp.tile([2 * H, B2], f16, name=f"h1_{g}", tag="h")
                    nc.scalar.activation(h1[:], ph[:], AF.Tanh, bias=b0[:])

                # -- x/u const halves fused into one matmul (rows 0:S = x,
                #    rows S:S+OUT = u), then the chain matmuls
                pxu = pxup.tile([32 + OUT, B], f32, name=f"pxu{t}", tag="pxu")
                nc.tensor.matmul(pxu[:], wxu[:], xy_h[:], start=True, stop=False)
                px = pxu[0:S, :]
                pu = pxu[32:, :]

                nc.tensor.matmul(p1[:], wdvw[:], w0[:], start=False, stop=True)
                if t < T - 1:
                    # x_next from the warm start w0: frees the boundary chain
                    nc.tensor.matmul(px, wxw[:], w0[:], start=False, stop=True,
                                     skip_group_check=True)
                    xy_hn = xyp.tile([S + IN, B], f16, name=f"xyh{t + 1}", tag="xy_h")
                    nc.sync.dma_start(xy_hn[0:IN, :], obsT16[t + 1])
                    nc.vector.tensor_add(xy_hn[IN:, :], px, xt_r[:])
                    if t < T - 2:
                        xt_rn = xtp.tile([S, B], f32, name=f"xtr{t + 1}", tag="xt_r")
                        nc.vector.tensor_add(xt_rn[:], px, xt_r[:])
                    else:
                        xt_rn = xt_r
                if u_last is not None and t > 0:
                    copy_u(t - 1)
                if g4 == 1:
                    ph2 = php.tile([2 * H, B2], f32, name=f"ph2_{g}", tag="ph")
                    nc.tensor.matmul(ph2[:], wv1b[:], h1[:], start=True, stop=True)

                # -- remaining fixed-point iterations (chain-paced).
                #    it>=2 accumulates D*(w_k - w_{k-1}) onto the same PSUM
                #    bank (p1 holds c + D*w_{k-1}): kills the const-refold MM.
                w_pp, w_prev = None, w0
                for it in range(1, n_iters):
                    if it > 1:
                        dl = wp.tile([NL, B], f16, name=f"d{t}_{it}", tag="d")
                        nc.vector.scalar_tensor_tensor(
                            dl[:], w_pp[:], -1.0, w_prev[:],
                            mybir.AluOpType.mult, mybir.AluOpType.add)
                        nc.tensor.matmul(p1[:], wdvw[:], dl[:], start=False,
                                         stop=True, skip_group_check=True)
                    wn = wp.tile([NL, B], f16, name=f"w{t}_{it}", tag="w")
                    nc.scalar.activation(wn[:], p1[:], AF.Tanh)
                    w_pp, w_prev = w_prev, wn
                    if it == 1 and g4 == 1:
                        h2 = vp.tile([2 * H, B2], f16, name=f"h2_{g}", tag="h")
                        nc.scalar.activation(h2[:], ph2[:], AF.Tanh, bias=b1[:])

                # defer u's wuw matmul (needs final w) into t+1's PE queue
                u_pend = (pu, w_prev, t)

                if g4 == 2:
                    pv = pvp.tile([2, B2], f32, name=f"pv_{g}", tag="pv")
                    nc.tensor.matmul(pv[:], wv2b[:], h2[:], start=True, stop=True)
                    v_sb = vp.tile([2, B2], f32, name=f"v_sb{g}", tag="v_sb")
                    nc.vector.tensor_copy(v_sb[:], pv[:])
                    nc.sync.dma_start(
                        v_out[t - 2:t + 2].rearrange("(r g1) b -> r (g1 b)", r=2),
                        v_sb[:])
                if g4 == VG - 1:
                    # rotate prefetched obs group
                    obs4 = obs4_next
                    if g + 2 < n_groups:
                        obs4_next = load_obs4(g + 2)

                if t < T - 1:
                    xt_r, xy_h = xt_rn, xy_hn

            # close out the final timestep's u
            pu_p, w_p, _ = u_pend
            nc.tensor.matmul(pu_p, wuw[:], w_p[:], start=False, stop=True,
                             skip_group_check=True)
            u_last = pu_p
            copy_u(T - 1)

    nc.compile()
    return nc


def host_inputs(inputs, core, n_cores=N_CORES):
    BL = inputs["obs"].shape[0] // n_cores
    sl = slice(core * BL, (core + 1) * BL)
    obs = np.ascontiguousarray(np.asarray(inputs["obs"])[sl].transpose(1, 2, 0))
    x0T = np.ascontiguousarray(np.asarray(inputs["x0"])[sl].T)
    g = lambda k: np.asarray(inputs[k]).astype(np.float32)
    Dvw = g("Dvw_T")
    M = np.linalg.inv(np.eye(NL, dtype=np.float32) - G_INIT * Dvw)
    Wcd = np.concatenate([g("Dvy_T"), g("Cv_T")], 0)
    W0, W1, W2 = g("W0"), g("W1"), g("W2")
    Z = np.zeros_like
    blk = lambda A: np.block([[A, Z(A)], [Z(A), A]])
    return {
        "obsT16": obs.astype(np.float16),
        "x0T": x0T.astype(np.float32),
        "Wdvw": Dvw.astype(np.float16),
        "Wcd": Wcd.astype(np.float16),
        "Wcd0": (Wcd @ M).astype(np.float16),
        "Wxu": np.concatenate(
            [np.concatenate([DT * g("By_T"), DT * g("A_T")], 0),
             np.zeros((S + IN, 32 - S), np.float32),
             np.concatenate([g("Duy_T"), g("Cu_T")], 0)], 1).astype(np.float16),
        "Wuw": g("Duw_T").astype(np.float16),
        "Wxw": (DT * g("Bw_T")).astype(np.float16),
        "Wv0b": blk(W0).astype(np.float16),
        "Wv1b": blk(W1).astype(np.float16),
        "Wv2b": blk(W2).astype(np.float16),
        "b0v": np.tile(g("b0").reshape(H, 1), (2, 1)).astype(np.float32),
        "b1v": np.tile(g("b1").reshape(H, 1), (2, 1)).astype(np.float32),
    }


def assemble_output(results, inputs, n_cores=N_CORES):
    obs = np.asarray(inputs["obs"])
    Bfull, T = obs.shape[0], obs.shape[1]
    BL = Bfull // n_cores
    out = np.empty((Bfull, T, 2 * OUT + 1), np.float32)
    log_stds = np.asarray(inputs["log_stds"], np.float32)
    b2 = np.asarray(inputs["b2"], np.float32)
    for c in range(n_cores):
        sl = slice(c * BL, (c + 1) * BL)
        out[sl, :, :OUT] = results[c]["u_out"].transpose(2, 0, 1)
        out[sl, :, OUT:2 * OUT] = log_stds
        out[sl, :, 2 * OUT:] = results[c]["v_out"].T[:, :, None] + b2
    return out


_NC_CACHE = {}


def _get_nc(T):
    if T not in _NC_CACHE:
        _NC_CACHE[T] = build_kernel(T=T)
    return _NC_CACHE[T]


def run_on_hw(inputs, trace=False):
    """Run the SPMD kernel; returns (full_output, exec_time_ns_or_None)."""
    T = np.asarray(inputs["obs"]).shape[1]
    nc = _get_nc(T)
    in_maps = [host_inputs(inputs, c) for c in range(N_CORES)]
    last_err = None
    for attempt in range(3):
        try:
            res = run_bass_kernel_spmd(nc, in_maps, list(range(N_CORES)), trace=trace)
            return assemble_output(res.results, inputs), res.exec_time_ns
        except Exception as e:  # transient device failures: retry
            last_err = e
    raise last_err


def kernel(**inputs) -> np.ndarray:
    out, _ = run_on_hw(inputs, trace=False)
    return out


# revision 15
# speedup vs baseline: 1.2097x; 1.2097x over previous
"""DissipativeThetaRINN Trainium2 (Bass/Tile) kernel — 8-core data parallel.

Strategy (pure data parallel, per sharding hint):
  - Batch B=2048 is split across 8 NeuronCores (256 rows/core); the tiny
    controller matrices and value-MLP weights are replicated.
  - On-device layout is transposed: features on SBUF partitions, batch on
    the free dimension (256 columns per core).
  - The implicit layer w = tanh(Cv x + Dvy y + Dvw w) is solved with only
    N_ITERS=3 tanh evaluations: iteration 0 uses a linear-solve warm start
    w0 = tanh(M c) with M = (I - g Dvw)^-1, g=0.8, folded host-side into
    the const matmul (zero extra device work).  Each remaining iteration is
    one 256-wide matmul pair into PSUM + one 256-wide tanh.  All matmuls
    span the full 256-column batch — on TRN2 the per-instruction overhead
    (LDWEIGHTS ~100ns + drain ~170ns) dominates 128-col streams, so fewer,
    wider instructions beat a 2-chunk ping-pong.
  - x_next (forward Euler) uses the second-to-last w iterate so the final
    tanh is off the timestep-boundary critical path (error ~1e-5, verified
    in fp16-faithful numpy sim: total rel_l2 ≈ 5.5e-3 vs 2e-2 budget).
  - Matmuls run in fp16 (PSUM accumulates fp32); DT is pre-folded into the
    recurrence weights; the x accumulator stays fp32 on device.
  - The value MLP is computed in groups of 4 timesteps, packed 2-per-128
    partitions with block-diagonal weights, scheduled into engine gaps.
  - log_stds broadcast and the +b2 value bias are applied host-side.
"""
import numpy as np
import concourse.bass as bass
import concourse.mybir as mybir
import concourse.tile as tile
from concourse import bacc
from concourse.bass_utils import run_bass_kernel_spmd

dt = mybir.dt
AF = mybir.ActivationFunctionType

# problem shape (hardcoded per contract)
BFULL, TFULL = 2048, 128
S, NL, IN, OUT, H = 16, 128, 32, 8, 64
DT = 0.01
N_CORES = 8
N_ITERS = 3    # tanh evaluations per timestep (incl. warm-start iteration)
G_INIT = 0.8   # warm-start gain: w0 = tanh((I - g Dvw)^-T c)
VG = 4         # value-MLP timestep group (packed 2x2 onto 128 partitions)


def build_kernel(T=TFULL, B=BFULL // N_CORES, n_iters=N_ITERS):
    nc = bacc.Bacc(None, target_bir_lowering=False)
    f32, f16 = dt.float32, dt.float16
    assert n_iters >= 2

    obsT16 = nc.dram_tensor("obsT16", [T, IN, B], f16, kind="ExternalInput")
    x0T = nc.dram_tensor("x0T", [S, B], f32, kind="ExternalInput")
    Wdvw = nc.dram_tensor("Wdvw", [NL, NL], f16, kind="ExternalInput")
    Wcd = nc.dram_tensor("Wcd", [S + IN, NL], f16, kind="ExternalInput")
    Wcd0 = nc.dram_tensor("Wcd0", [S + IN, NL], f16, kind="ExternalInput")
    Wxu = nc.dram_tensor("Wxu", [S + IN, 32 + OUT], f16, kind="ExternalInput")
    Wuw = nc.dram_tensor("Wuw", [NL, OUT], f16, kind="ExternalInput")
    Wxw = nc.dram_tensor("Wxw", [NL, S], f16, kind="ExternalInput")
    Wv0b = nc.dram_tensor("Wv0b", [2 * IN, 2 * H], f16, kind="ExternalInput")
    Wv1b = nc.dram_tensor("Wv1b", [2 * H, 2 * H], f16, kind="ExternalInput")
    Wv2b = nc.dram_tensor("Wv2b", [2 * H, 2], f16, kind="ExternalInput")
    b0v = nc.dram_tensor("b0v", [2 * H, 1], f32, kind="ExternalInput")
    b1v = nc.dram_tensor("b1v", [2 * H, 1], f32, kind="ExternalInput")

    u_out = nc.dram_tensor("u_out", [T, OUT, B], f32, kind="ExternalOutput")
    v_out = nc.dram_tensor("v_out", [T, B], f32, kind="ExternalOutput")

    B2 = 2 * B
    n_groups = (T + VG - 1) // VG

    with tile.TileContext(nc) as tc:
        with tc.tile_pool(name="wts", bufs=1) as wts, \
             tc.tile_pool(name="xyp", bufs=3) as xyp, \
             tc.tile_pool(name="xtp", bufs=2) as xtp, \
             tc.tile_pool(name="wp", bufs=3) as wp, \
             tc.tile_pool(name="obp", bufs=2) as obp, \
             tc.tile_pool(name="vp", bufs=2) as vp, \
             tc.tile_pool(name="up", bufs=2) as up, \
             tc.tile_pool(name="pw", bufs=2, space="PSUM") as pwp, \
             tc.tile_pool(name="pwc", bufs=2, space="PSUM") as pwcp, \
             tc.tile_pool(name="pxu", bufs=2, space="PSUM") as pxup, \
             tc.tile_pool(name="ph", bufs=1, space="PSUM") as php, \
             tc.tile_pool(name="pv", bufs=1, space="PSUM") as pvp:

            def wt(name, dram, shape, dtp):
                tl = wts.tile(shape, dtp, name=name)
                nc.sync.dma_start(tl[:], dram[:])
                return tl
            wdvw = wt("wdvw", Wdvw, [NL, NL], f16)
            wcd = wt("wcd", Wcd, [S + IN, NL], f16)
            wcd0 = wt("wcd0", Wcd0, [S + IN, NL], f16)
            wxu = wt("wxu", Wxu, [S + IN, 32 + OUT], f16)
            wuw = wt("wuw", Wuw, [NL, OUT], f16)
            wxw = wt("wxw", Wxw, [NL, S], f16)
            wv0b = wt("wv0b", Wv0b, [2 * IN, 2 * H], f16)
            wv1b = wt("wv1b", Wv1b, [2 * H, 2 * H], f16)
            wv2b = wt("wv2b", Wv2b, [2 * H, 2], f16)
            b0 = wt("b0", b0v, [2 * H, 1], f32)
            b1 = wt("b1", b1v, [2 * H, 1], f32)

            def load_obs4(g):
                """obs4 [64, 2B]: rows 0:32 = ts {4g, 4g+1}, rows 32:64 =
                ts {4g+2, 4g+3} (two col blocks of B each)."""
                t0 = g * VG
                ob = obp.tile([2 * IN, B2], f16, name=f"obs4_{g}", tag="obs4")
                nc.sync.dma_start(
                    ob[0:IN, :].rearrange("k (g1 b) -> k g1 b", g1=2),
                    obsT16[t0:t0 + 2].transpose([1, 0, 2]))
                nc.sync.dma_start(
                    ob[IN:, :].rearrange("k (g1 b) -> k g1 b", g1=2),
                    obsT16[t0 + 2:t0 + 4].transpose([1, 0, 2]))
                return ob

            obs4 = load_obs4(0)
            obs4_next = load_obs4(1) if n_groups > 1 else None

            # xy_h [48, B] f16: rows 0:32 = y^T, rows 32:48 = x^T
            xt_r = xtp.tile([S, B], f32, name="xt_r0", tag="xt_r")
            nc.sync.dma_start(xt_r[:], x0T[:])
            xy_h = xyp.tile([S + IN, B], f16, name="xy_h0", tag="xy_h")
            nc.sync.dma_start(xy_h[0:IN, :], obsT16[0])
            nc.vector.tensor_copy(xy_h[IN:, :], xt_r[:])

            u4 = None
            u_pend = None  # (pu_tile, w_final, t): wuw matmul deferred to t+1
            u_last = None

            def copy_u(tp):
                nonlocal u4
                g4p = tp % VG
                if g4p == 0:
                    u4 = up.tile([OUT, VG * B], f32, name=f"u4_{tp // VG}", tag="u4")
                nc.vector.tensor_copy(u4[:, g4p * B:(g4p + 1) * B], u_last)
                if g4p == VG - 1:
                    nc.sync.dma_start(
                        u_out[tp - VG + 1:tp + 1].transpose([1, 0, 2]),
                        u4[:].rearrange("o (g1 b) -> o g1 b", g1=VG))

            for t in range(T):
                g, g4 = t // VG, t % VG
                # -- PE front: warm-start + it1-const (need only xy), value
                #    layer-0, then t-1's deferred u close.
                p0 = pwp.tile([NL, B], f32, name=f"pw{t}_0", tag="pw")
                nc.tensor.matmul(p0[:], wcd0[:], xy_h[:], start=True, stop=True)
                pc = pwcp.tile([NL, 2 * B], f32, name=f"pc{t}", tag="pc")
                xy_rep = xy_h[:].unsqueeze(1).broadcast_to([S + IN, 2, B])
                nc.tensor.matmul(pc[:], wcd[:], xy_rep, start=True, stop=False,
                                 skip_group_check=True)
                if g4 == 0:
                    ph = php.tile([2 * H, B2], f32, name=f"ph_{g}", tag="ph")
                    nc.tensor.matmul(ph[:], wv0b[:], obs4[:], start=True, stop=True)
                if u_pend is not None:
                    pu_p, w_p, _ = u_pend
                    nc.tensor.matmul(pu_p, wuw[:], w_p[:], start=False, stop=True,
                                     skip_group_check=True)
                    u_last, u_pend = pu_p, None

                w0 = wp.tile([NL, B], f16, name=f"w{t}_0", tag="w")
                nc.scalar.activation(w0[:], p0[:], AF.Tanh)           # tanh0
                if g4 == 0:
                    h1 = vp.tile([2 * H, B2], f16, name=f"h1_{g}", tag="h")
                    nc.scalar.activation(h1[:], ph[:], AF.Tanh, bias=b0[:])

                # -- x/u const halves fused into one matmul (rows 0:S = x,
                #    rows S:S+OUT = u), then the chain matmuls
                pxu = pxup.tile([32 + OUT, B], f32, name=f"pxu{t}", tag="pxu")
                nc.tensor.matmul(pxu[:], wxu[:], xy_h[:], start=True, stop=False)
                px = pxu[0:S, :]
                pu = pxu[32:, :]

                nc.tensor.matmul(pc[:, 0:B], wdvw[:], w0[:], start=False,
                                 stop=True, skip_group_check=True)
                if t < T - 1:
                    # x_next from the warm start w0: frees the boundary chain
                    nc.tensor.matmul(px, wxw[:], w0[:], start=False, stop=True,
                                     skip_group_check=True)
                    xy_hn = xyp.tile([S + IN, B], f16, name=f"xyh{t + 1}", tag="xy_h")
                    nc.sync.dma_start(xy_hn[0:IN, :], obsT16[t + 1])
                    nc.vector.tensor_add(xy_hn[IN:, :], px, xt_r[:])
                    if t < T - 2:
                        xt_rn = xtp.tile([S, B], f32, name=f"xtr{t + 1}", tag="xt_r")
                        nc.vector.tensor_add(xt_rn[:], px, xt_r[:])
                    else:
                        xt_rn = xt_r
                if u_last is not None and t > 0:
                    copy_u(t - 1)
                if g4 == 1:
                    ph2 = php.tile([2 * H, B2], f32, name=f"ph2_{g}", tag="ph")
                    nc.tensor.matmul(ph2[:], wv1b[:], h1[:], start=True, stop=True)

                # -- remaining fixed-point iterations (chain-paced); the
                #    replicated const tile pc = [c|c] supplies each iteration's
                #    refold half, so only the wdvw matmul is per-iteration.
                w_prev = w0
                for it in range(1, n_iters):
                    half = slice((it - 1) * B, it * B)
                    if it > 1:
                        nc.tensor.matmul(pc[:, half], wdvw[:], w_prev[:],
                                         start=False, stop=True,
                                         skip_group_check=True)
                    wn = wp.tile([NL, B], f16, name=f"w{t}_{it}", tag="w")
                    nc.scalar.activation(wn[:], pc[:, half], AF.Tanh)
                    w_prev = wn
                    if it == 1 and g4 == 1:
                        h2 = vp.tile([2 * H, B2], f16, name=f"h2_{g}", tag="h")
                        nc.scalar.activation(h2[:], ph2[:], AF.Tanh, bias=b1[:])

                # defer u's wuw matmul (needs final w) into t+1's PE queue
                u_pend = (pu, w_prev, t)

                if g4 == 2:
                    pv = pvp.tile([2, B2], f32, name=f"pv_{g}", tag="pv")
                    nc.tensor.matmul(pv[:], wv2b[:], h2[:], start=True, stop=True)
                    v_sb = vp.tile([2, B2], f32, name=f"v_sb{g}", tag="v_sb")
                    nc.vector.tensor_copy(v_sb[:], pv[:])
                    nc.sync.dma_start(
                        v_out[t - 2:t + 2].rearrange("(r g1) b -> r (g1 b)", r=2),
                        v_sb[:])
                if g4 == VG - 1:
                    # rotate prefetched obs group
                    obs4 = obs4_next
                    if g + 2 < n_groups:
                        obs4_next = load_obs4(g + 2)

                if t < T - 1:
                    xt_r, xy_h = xt_rn, xy_hn

            # close out the final timestep's u
            pu_p, w_p, _ = u_pend
            nc.tensor.matmul(pu_p, wuw[:], w_p[:], start=False, stop=True,
                             skip_group_check=True)
            u_last = pu_p
            copy_u(T - 1)

    nc.compile()
    return nc


def host_inputs(inputs, core, n_cores=N_CORES):
    BL = inputs["obs"].shape[0] // n_cores
    sl = slice(core * BL, (core + 1) * BL)
    obs = np.ascontiguousarray(np.asarray(inputs["obs"])[sl].transpose(1, 2, 0))
    x0T = np.ascontiguousarray(np.asarray(inputs["x0"])[sl].T)
    g = lambda k: np.asarray(inputs[k]).astype(np.float32)
    Dvw = g("Dvw_T")
    M = np.linalg.inv(np.eye(NL, dtype=np.float32) - G_INIT * Dvw)
    Wcd = np.concatenate([g("Dvy_T"), g("Cv_T")], 0)
    W0, W1, W2 = g("W0"), g("W1"), g("W2")
    Z = np.zeros_like
    blk = lambda A: np.block([[A, Z(A)], [Z(A), A]])
    return {
        "obsT16": obs.astype(np.float16),
        "x0T": x0T.astype(np.float32),
        "Wdvw": Dvw.astype(np.float16),
        "Wcd": Wcd.astype(np.float16),
        "Wcd0": (Wcd @ M).astype(np.float16),
        "Wxu": np.concatenate(
            [np.concatenate([DT * g("By_T"), DT * g("A_T")], 0),
             np.zeros((S + IN, 32 - S), np.float32),
             np.concatenate([g("Duy_T"), g("Cu_T")], 0)], 1).astype(np.float16),
        "Wuw": g("Duw_T").astype(np.float16),
        "Wxw": (DT * g("Bw_T")).astype(np.float16),
        "Wv0b": blk(W0).astype(np.float16),
        "Wv1b": blk(W1).astype(np.float16),
        "Wv2b": blk(W2).astype(np.float16),
        "b0v": np.tile(g("b0").reshape(H, 1), (2, 1)).astype(np.float32),
        "b1v": np.tile(g("b1").reshape(H, 1), (2, 1)).astype(np.float32),
    }


def assemble_output(results, inputs, n_cores=N_CORES):
    obs = np.asarray(inputs["obs"])
    Bfull, T = obs.shape[0], obs.shape[1]
    BL = Bfull // n_cores
    out = np.empty((Bfull, T, 2 * OUT + 1), np.float32)
    log_stds = np.asarray(inputs["log_stds"], np.float32)
    b2 = np.asarray(inputs["b2"], np.float32)
    for c in range(n_cores):
        sl = slice(c * BL, (c + 1) * BL)
        out[sl, :, :OUT] = results[c]["u_out"].transpose(2, 0, 1)
        out[sl, :, OUT:2 * OUT] = log_stds
        out[sl, :, 2 * OUT:] = results[c]["v_out"].T[:, :, None] + b2
    return out


_NC_CACHE = {}


def _get_nc(T):
    if T not in _NC_CACHE:
        _NC_CACHE[T] = build_kernel(T=T)
    return _NC_CACHE[T]


def run_on_hw(inputs, trace=False):
    """Run the SPMD kernel; returns (full_output, exec_time_ns_or_None)."""
    T = np.asarray(inputs["obs"]).shape[1]
    nc = _get_nc(T)
    in_maps = [host_inputs(inputs, c) for c in range(N_CORES)]
    last_err = None
    for attempt in range(3):
        try:
            res = run_bass_kernel_spmd(nc, in_maps, list(range(N_CORES)), trace=trace)
            return assemble_output(res.results, inputs), res.exec_time_ns
        except Exception as e:  # transient device failures: retry
            last_err = e
    raise last_err


def kernel(**inputs) -> np.ndarray:
    out, _ = run_on_hw(inputs, trace=False)
    return out


# revision 16
# speedup vs baseline: 1.3555x; 1.1205x over previous
"""DissipativeThetaRINN Trainium2 (Bass/Tile) kernel — 8-core data parallel.

Strategy (pure data parallel, per sharding hint):
  - Batch B=2048 is split across 8 NeuronCores (256 rows/core); the tiny
    controller matrices and value-MLP weights are replicated.
  - On-device layout is transposed: features on SBUF partitions, batch on
    the free dimension (256 columns per core).
  - The implicit layer w = tanh(Cv x + Dvy y + Dvw w) is solved with only
    N_ITERS=3 tanh evaluations: iteration 0 uses a linear-solve warm start
    w0 = tanh(M c) with M = (I - g Dvw)^-1, g=0.8, folded host-side into
    the const matmul (zero extra device work).  Each remaining iteration is
    one 256-wide matmul pair into PSUM + one 256-wide tanh.  All matmuls
    span the full 256-column batch — on TRN2 the per-instruction overhead
    (LDWEIGHTS ~100ns + drain ~170ns) dominates 128-col streams, so fewer,
    wider instructions beat a 2-chunk ping-pong.
  - x_next (forward Euler) uses the second-to-last w iterate so the final
    tanh is off the timestep-boundary critical path (error ~1e-5, verified
    in fp16-faithful numpy sim: total rel_l2 ≈ 5.5e-3 vs 2e-2 budget).
  - Matmuls run in fp16 (PSUM accumulates fp32); DT is pre-folded into the
    recurrence weights; the x accumulator stays fp32 on device.
  - The value MLP is computed in groups of 4 timesteps, packed 2-per-128
    partitions with block-diagonal weights, scheduled into engine gaps.
  - log_stds broadcast and the +b2 value bias are applied host-side.
"""
import numpy as np
import concourse.bass as bass
import concourse.mybir as mybir
import concourse.tile as tile
from concourse import bacc
from concourse.bass_utils import run_bass_kernel_spmd

dt = mybir.dt
AF = mybir.ActivationFunctionType

# problem shape (hardcoded per contract)
BFULL, TFULL = 2048, 128
S, NL, IN, OUT, H = 16, 128, 32, 8, 64
DT = 0.01
N_CORES = 8
N_ITERS = 2    # tanh evaluations per timestep (incl. warm-start iteration)
G_INIT = 0.8   # warm-start gain: w0 = tanh((I - g Dvw)^-T c)
GK = 0.75      # u-readout correction: u += (w1-w0) @ ((I-gK D)^-1 - I) @ Duw
VG = 4         # value-MLP timestep group (packed 2x2 onto 128 partitions)


def build_kernel(T=TFULL, B=BFULL // N_CORES, n_iters=N_ITERS):
    nc = bacc.Bacc(None, target_bir_lowering=False)
    f32, f16 = dt.float32, dt.float16
    assert n_iters >= 2

    obsT16 = nc.dram_tensor("obsT16", [T, IN, B], f16, kind="ExternalInput")
    x0T = nc.dram_tensor("x0T", [S, B], f32, kind="ExternalInput")
    Wdvw = nc.dram_tensor("Wdvw", [NL, NL], f16, kind="ExternalInput")
    Wcd = nc.dram_tensor("Wcd", [S + IN, NL], f16, kind="ExternalInput")
    Wcd0 = nc.dram_tensor("Wcd0", [S + IN, NL], f16, kind="ExternalInput")
    Wxu = nc.dram_tensor("Wxu", [S + IN, 32 + OUT], f16, kind="ExternalInput")
    Wuw = nc.dram_tensor("Wuw", [NL, OUT], f16, kind="ExternalInput")
    Wxuw = nc.dram_tensor("Wxuw", [NL, 32 + OUT], f16, kind="ExternalInput")
    Wv0b = nc.dram_tensor("Wv0b", [2 * IN, 2 * H], f16, kind="ExternalInput")
    Wv1b = nc.dram_tensor("Wv1b", [2 * H, 2 * H], f16, kind="ExternalInput")
    Wv2b = nc.dram_tensor("Wv2b", [2 * H, 2], f16, kind="ExternalInput")
    b0v = nc.dram_tensor("b0v", [2 * H, 1], f32, kind="ExternalInput")
    b1v = nc.dram_tensor("b1v", [2 * H, 1], f32, kind="ExternalInput")

    u_out = nc.dram_tensor("u_out", [T, OUT, B], f32, kind="ExternalOutput")
    v_out = nc.dram_tensor("v_out", [T, B], f32, kind="ExternalOutput")

    B2 = 2 * B
    n_groups = (T + VG - 1) // VG

    with tile.TileContext(nc) as tc:
        with tc.tile_pool(name="wts", bufs=1) as wts, \
             tc.tile_pool(name="xyp", bufs=3) as xyp, \
             tc.tile_pool(name="xtp", bufs=2) as xtp, \
             tc.tile_pool(name="wp", bufs=3) as wp, \
             tc.tile_pool(name="obp", bufs=2) as obp, \
             tc.tile_pool(name="vp", bufs=2) as vp, \
             tc.tile_pool(name="up", bufs=2) as up, \
             tc.tile_pool(name="pw", bufs=2, space="PSUM") as pwp, \
             tc.tile_pool(name="pxu", bufs=2, space="PSUM") as pxup, \
             tc.tile_pool(name="ph", bufs=1, space="PSUM") as php, \
             tc.tile_pool(name="pv", bufs=1, space="PSUM") as pvp:

            def wt(name, dram, shape, dtp):
                tl = wts.tile(shape, dtp, name=name)
                nc.sync.dma_start(tl[:], dram[:])
                return tl
            wdvw = wt("wdvw", Wdvw, [NL, NL], f16)
            wcd = wt("wcd", Wcd, [S + IN, NL], f16)
            wcd0 = wt("wcd0", Wcd0, [S + IN, NL], f16)
            wxu = wt("wxu", Wxu, [S + IN, 32 + OUT], f16)
            wuw = wt("wuw", Wuw, [NL, OUT], f16)
            wxuw = wt("wxuw", Wxuw, [NL, 32 + OUT], f16)
            wv0b = wt("wv0b", Wv0b, [2 * IN, 2 * H], f16)
            wv1b = wt("wv1b", Wv1b, [2 * H, 2 * H], f16)
            wv2b = wt("wv2b", Wv2b, [2 * H, 2], f16)
            b0 = wt("b0", b0v, [2 * H, 1], f32)
            b1 = wt("b1", b1v, [2 * H, 1], f32)

            def load_obs4(g):
                """obs4 [64, 2B]: rows 0:32 = ts {4g, 4g+1}, rows 32:64 =
                ts {4g+2, 4g+3} (two col blocks of B each)."""
                t0 = g * VG
                ob = obp.tile([2 * IN, B2], f16, name=f"obs4_{g}", tag="obs4")
                nc.sync.dma_start(
                    ob[0:IN, :].rearrange("k (g1 b) -> k g1 b", g1=2),
                    obsT16[t0:t0 + 2].transpose([1, 0, 2]))
                nc.sync.dma_start(
                    ob[IN:, :].rearrange("k (g1 b) -> k g1 b", g1=2),
                    obsT16[t0 + 2:t0 + 4].transpose([1, 0, 2]))
                return ob

            obs4 = load_obs4(0)
            obs4_next = load_obs4(1) if n_groups > 1 else None

            # xy_h [48, B] f16: rows 0:32 = y^T, rows 32:48 = x^T
            xt_r = xtp.tile([S, B], f32, name="xt_r0", tag="xt_r")
            nc.sync.dma_start(xt_r[:], x0T[:])
            xy_h = xyp.tile([S + IN, B], f16, name="xy_h0", tag="xy_h")
            nc.sync.dma_start(xy_h[0:IN, :], obsT16[0])
            nc.vector.tensor_copy(xy_h[IN:, :], xt_r[:])

            u4 = None
            u_pend = None  # (pu_tile, w_final, t): wuw matmul deferred to t+1
            u_last = None

            def copy_u(tp):
                nonlocal u4
                g4p = tp % VG
                if g4p == 0:
                    u4 = up.tile([OUT, VG * B], f32, name=f"u4_{tp // VG}", tag="u4")
                nc.vector.tensor_copy(u4[:, g4p * B:(g4p + 1) * B], u_last)
                if g4p == VG - 1:
                    nc.sync.dma_start(
                        u_out[tp - VG + 1:tp + 1].transpose([1, 0, 2]),
                        u4[:].rearrange("o (g1 b) -> o g1 b", g1=VG))

            for t in range(T):
                g, g4 = t // VG, t % VG
                # -- PE front: warm-start + it1-const (need only xy), value
                #    layer-0, then t-1's deferred u close.
                p0 = pwp.tile([NL, B], f32, name=f"pw{t}_0", tag="pw")
                nc.tensor.matmul(p0[:], wcd0[:], xy_h[:], start=True, stop=True)
                p1 = pwp.tile([NL, B], f32, name=f"pw{t}_1", tag="pw")
                nc.tensor.matmul(p1[:], wcd[:], xy_h[:], start=True, stop=False)
                if g4 == 0:
                    ph = php.tile([2 * H, B2], f32, name=f"ph_{g}", tag="ph")
                    nc.tensor.matmul(ph[:], wv0b[:], obs4[:], start=True, stop=True)
                if u_pend is not None:
                    pu_p, w_p, _ = u_pend
                    nc.tensor.matmul(pu_p, wuw[:], w_p[:], start=False, stop=True,
                                     skip_group_check=True)
                    u_last, u_pend = pu_p, None

                w0 = wp.tile([NL, B], f16, name=f"w{t}_0", tag="w")
                nc.scalar.activation(w0[:], p0[:], AF.Tanh)           # tanh0
                if g4 == 0:
                    h1 = vp.tile([2 * H, B2], f16, name=f"h1_{g}", tag="h")
                    nc.scalar.activation(h1[:], ph[:], AF.Tanh, bias=b0[:])

                # -- x/u const halves fused into one matmul (rows 0:S = x,
                #    rows S:S+OUT = u), then the chain matmuls
                pxu = pxup.tile([32 + OUT, B], f32, name=f"pxu{t}", tag="pxu")
                nc.tensor.matmul(pxu[:], wxu[:], xy_h[:], start=True, stop=False)
                px = pxu[0:S, :]
                pu = pxu[32:, :]

                nc.tensor.matmul(p1[:], wdvw[:], w0[:], start=False, stop=True)
                # w0-side terms: x_next rows 0:S and u's -K*Duw rows 32:40,
                # one matmul (x_next from the warm start frees the boundary)
                nc.tensor.matmul(pxu[:], wxuw[:], w0[:], start=False, stop=False,
                                 skip_group_check=True)
                if t < T - 1:
                    xy_hn = xyp.tile([S + IN, B], f16, name=f"xyh{t + 1}", tag="xy_h")
                    nc.sync.dma_start(xy_hn[0:IN, :], obsT16[t + 1])
                    nc.vector.tensor_add(xy_hn[IN:, :], px, xt_r[:])
                    if t < T - 2:
                        xt_rn = xtp.tile([S, B], f32, name=f"xtr{t + 1}", tag="xt_r")
                        nc.vector.tensor_add(xt_rn[:], px, xt_r[:])
                    else:
                        xt_rn = xt_r
                if u_last is not None and t > 0:
                    copy_u(t - 1)
                if g4 == 1:
                    ph2 = php.tile([2 * H, B2], f32, name=f"ph2_{g}", tag="ph")
                    nc.tensor.matmul(ph2[:], wv1b[:], h1[:], start=True, stop=True)

                # -- final refold iteration
                w1 = wp.tile([NL, B], f16, name=f"w{t}_1", tag="w")
                nc.scalar.activation(w1[:], p1[:], AF.Tanh)
                w_prev = w1
                if g4 == 1:
                    h2 = vp.tile([2 * H, B2], f16, name=f"h2_{g}", tag="h")
                    nc.scalar.activation(h2[:], ph2[:], AF.Tanh, bias=b1[:])

                # defer u's wuw matmul (needs final w) into t+1's PE queue
                u_pend = (pu, w_prev, t)

                if g4 == 2:
                    pv = pvp.tile([2, B2], f32, name=f"pv_{g}", tag="pv")
                    nc.tensor.matmul(pv[:], wv2b[:], h2[:], start=True, stop=True)
                    v_sb = vp.tile([2, B2], f32, name=f"v_sb{g}", tag="v_sb")
                    nc.vector.tensor_copy(v_sb[:], pv[:])
                    nc.sync.dma_start(
                        v_out[t - 2:t + 2].rearrange("(r g1) b -> r (g1 b)", r=2),
                        v_sb[:])
                if g4 == VG - 1:
                    # rotate prefetched obs group
                    obs4 = obs4_next
                    if g + 2 < n_groups:
                        obs4_next = load_obs4(g + 2)

                if t < T - 1:
                    xt_r, xy_h = xt_rn, xy_hn

            # close out the final timestep's u
            pu_p, w_p, _ = u_pend
            nc.tensor.matmul(pu_p, wuw[:], w_p[:], start=False, stop=True,
                             skip_group_check=True)
            u_last = pu_p
            copy_u(T - 1)

    nc.compile()
    return nc


def host_inputs(inputs, core, n_cores=N_CORES):
    BL = inputs["obs"].shape[0] // n_cores
    sl = slice(core * BL, (core + 1) * BL)
    obs = np.ascontiguousarray(np.asarray(inputs["obs"])[sl].transpose(1, 2, 0))
    x0T = np.ascontiguousarray(np.asarray(inputs["x0"])[sl].T)
    g = lambda k: np.asarray(inputs[k]).astype(np.float32)
    Dvw = g("Dvw_T")
    M = np.linalg.inv(np.eye(NL, dtype=np.float32) - G_INIT * Dvw)
    KI = np.linalg.inv(np.eye(NL, dtype=np.float32) - GK * Dvw)  # = I + K
    Wcd = np.concatenate([g("Dvy_T"), g("Cv_T")], 0)
    W0, W1, W2 = g("W0"), g("W1"), g("W2")
    Z = np.zeros_like
    blk = lambda A: np.block([[A, Z(A)], [Z(A), A]])
    return {
        "obsT16": obs.astype(np.float16),
        "x0T": x0T.astype(np.float32),
        "Wdvw": Dvw.astype(np.float16),
        "Wcd": Wcd.astype(np.float16),
        "Wcd0": (Wcd @ M).astype(np.float16),
        "Wxu": np.concatenate(
            [np.concatenate([DT * g("By_T"), DT * g("A_T")], 0),
             np.zeros((S + IN, 32 - S), np.float32),
             np.concatenate([g("Duy_T"), g("Cu_T")], 0)], 1).astype(np.float16),
        "Wuw": (KI @ g("Duw_T")).astype(np.float16),
        "Wxuw": np.concatenate(
            [DT * g("Bw_T"), np.zeros((NL, 32 - S), np.float32),
             -(KI - np.eye(NL, dtype=np.float32)) @ g("Duw_T")], 1
        ).astype(np.float16),
        "Wv0b": blk(W0).astype(np.float16),
        "Wv1b": blk(W1).astype(np.float16),
        "Wv2b": blk(W2).astype(np.float16),
        "b0v": np.tile(g("b0").reshape(H, 1), (2, 1)).astype(np.float32),
        "b1v": np.tile(g("b1").reshape(H, 1), (2, 1)).astype(np.float32),
    }


def assemble_output(results, inputs, n_cores=N_CORES):
    obs = np.asarray(inputs["obs"])
    Bfull, T = obs.shape[0], obs.shape[1]
    BL = Bfull // n_cores
    out = np.empty((Bfull, T, 2 * OUT + 1), np.float32)
    log_stds = np.asarray(inputs["log_stds"], np.float32)
    b2 = np.asarray(inputs["b2"], np.float32)
    for c in range(n_cores):
        sl = slice(c * BL, (c + 1) * BL)
        out[sl, :, :OUT] = results[c]["u_out"].transpose(2, 0, 1)
        out[sl, :, OUT:2 * OUT] = log_stds
        out[sl, :, 2 * OUT:] = results[c]["v_out"].T[:, :, None] + b2
    return out


_NC_CACHE = {}


def _get_nc(T):
    if T not in _NC_CACHE:
        _NC_CACHE[T] = build_kernel(T=T)
    return _NC_CACHE[T]


def run_on_hw(inputs, trace=False):
    """Run the SPMD kernel; returns (full_output, exec_time_ns_or_None)."""
    T = np.asarray(inputs["obs"]).shape[1]
    nc = _get_nc(T)
    in_maps = [host_inputs(inputs, c) for c in range(N_CORES)]
    last_err = None
    for attempt in range(3):
        try:
            res = run_bass_kernel_spmd(nc, in_maps, list(range(N_CORES)), trace=trace)
            return assemble_output(res.results, inputs), res.exec_time_ns
        except Exception as e:  # transient device failures: retry
            last_err = e
    raise last_err


def kernel(**inputs) -> np.ndarray:
    out, _ = run_on_hw(inputs, trace=False)
    return out


# revision 18
# speedup vs baseline: 1.3880x; 1.0240x over previous
"""DissipativeThetaRINN Trainium2 (Bass/Tile) kernel — 8-core data parallel.

Strategy (pure data parallel, per sharding hint):
  - Batch B=2048 is split across 8 NeuronCores (256 rows/core); the tiny
    controller matrices and value-MLP weights are replicated.
  - On-device layout is transposed: features on SBUF partitions, batch on
    the free dimension (256 columns per core).
  - The implicit layer w = tanh(Cv x + Dvy y + Dvw w) is solved with only
    N_ITERS=2 tanh evaluations: iteration 0 uses a linear-solve warm start
    w0 = tanh(M c) with M = (I - g Dvw)^-1, g=0.8, folded host-side into
    the const matmul (zero extra device work); one refold gives w1.  The
    u readout then applies a linear-solve correction for the remaining
    fixed-point residual: u += (w1 - w0) @ ((I - gK Dvw)^-1 - I) @ Duw,
    gK=0.75, folded host-side into the two u weight matrices (the w0-side
    term rides the x-update matmul).  All matmuls span the full 256-column
    batch — on TRN2 the per-instruction overhead (LDWEIGHTS ~100ns +
    drain ~170ns) dominates 128-col streams, so fewer, wider instructions
    beat a 2-chunk ping-pong.  Verified in a fp16-faithful numpy sim:
    rel_l2 = 4.6e-3 vs the 2e-2 budget.
  - x_next (forward Euler) uses the second-to-last w iterate so the final
    tanh is off the timestep-boundary critical path (error ~1e-5, verified
    in fp16-faithful numpy sim: total rel_l2 ≈ 5.5e-3 vs 2e-2 budget).
  - Matmuls run in fp16 (PSUM accumulates fp32); DT is pre-folded into the
    recurrence weights; the x accumulator stays fp32 on device.
  - The value MLP is computed in groups of 4 timesteps, packed 2-per-128
    partitions with block-diagonal weights, scheduled into engine gaps.
  - log_stds broadcast and the +b2 value bias are applied host-side.
"""
import numpy as np
import concourse.bass as bass
import concourse.mybir as mybir
import concourse.tile as tile
from concourse import bacc
from concourse.bass_utils import run_bass_kernel_spmd

dt = mybir.dt
AF = mybir.ActivationFunctionType

# problem shape (hardcoded per contract)
BFULL, TFULL = 2048, 128
S, NL, IN, OUT, H = 16, 128, 32, 8, 64
DT = 0.01
N_CORES = 8
N_ITERS = 2    # tanh evaluations per timestep (incl. warm-start iteration)
G_INIT = 0.8   # warm-start gain: w0 = tanh((I - g Dvw)^-T c)
GK = 0.75      # u-readout correction: u += (w1-w0) @ ((I-gK D)^-1 - I) @ Duw
VG = 4         # value-MLP timestep group (packed 2x2 onto 128 partitions)


def build_kernel(T=TFULL, B=BFULL // N_CORES, n_iters=N_ITERS):
    nc = bacc.Bacc(None, target_bir_lowering=False)
    f32, f16 = dt.float32, dt.float16
    assert n_iters >= 2

    obsT16 = nc.dram_tensor("obsT16", [T, IN, B], f16, kind="ExternalInput")
    x0T = nc.dram_tensor("x0T", [S, B], f32, kind="ExternalInput")
    Wdvw = nc.dram_tensor("Wdvw", [NL, NL], f16, kind="ExternalInput")
    Wcd = nc.dram_tensor("Wcd", [S + IN, NL], f16, kind="ExternalInput")
    Wcd0 = nc.dram_tensor("Wcd0", [S + IN, NL], f16, kind="ExternalInput")
    Wxu = nc.dram_tensor("Wxu", [S + IN, 32 + OUT], f16, kind="ExternalInput")
    Wuw = nc.dram_tensor("Wuw", [NL, OUT], f16, kind="ExternalInput")
    Wxuw = nc.dram_tensor("Wxuw", [NL, 32 + OUT], f16, kind="ExternalInput")
    Wv0b = nc.dram_tensor("Wv0b", [2 * IN, 2 * H], f16, kind="ExternalInput")
    Wv1b = nc.dram_tensor("Wv1b", [2 * H, 2 * H], f16, kind="ExternalInput")
    Wv2b = nc.dram_tensor("Wv2b", [2 * H, 2], f16, kind="ExternalInput")
    b0v = nc.dram_tensor("b0v", [2 * H, 1], f32, kind="ExternalInput")
    b1v = nc.dram_tensor("b1v", [2 * H, 1], f32, kind="ExternalInput")

    u_out = nc.dram_tensor("u_out", [T, OUT, B], f32, kind="ExternalOutput")
    v_out = nc.dram_tensor("v_out", [T, B], f32, kind="ExternalOutput")

    B2 = 2 * B
    n_groups = (T + VG - 1) // VG

    with tile.TileContext(nc) as tc:
        with tc.tile_pool(name="wts", bufs=1) as wts, \
             tc.tile_pool(name="xyp", bufs=3) as xyp, \
             tc.tile_pool(name="xtp", bufs=2) as xtp, \
             tc.tile_pool(name="wp", bufs=3) as wp, \
             tc.tile_pool(name="obp", bufs=2) as obp, \
             tc.tile_pool(name="vp", bufs=2) as vp, \
             tc.tile_pool(name="up", bufs=2) as up, \
             tc.tile_pool(name="pw", bufs=2, space="PSUM") as pwp, \
             tc.tile_pool(name="pxu", bufs=2, space="PSUM") as pxup, \
             tc.tile_pool(name="ph", bufs=1, space="PSUM") as php, \
             tc.tile_pool(name="pv", bufs=1, space="PSUM") as pvp:

            def wt(name, dram, shape, dtp):
                tl = wts.tile(shape, dtp, name=name)
                nc.sync.dma_start(tl[:], dram[:])
                return tl
            wdvw = wt("wdvw", Wdvw, [NL, NL], f16)
            wcd = wt("wcd", Wcd, [S + IN, NL], f16)
            wcd0 = wt("wcd0", Wcd0, [S + IN, NL], f16)
            wxu = wt("wxu", Wxu, [S + IN, 32 + OUT], f16)
            wuw = wt("wuw", Wuw, [NL, OUT], f16)
            wxuw = wt("wxuw", Wxuw, [NL, 32 + OUT], f16)
            wv0b = wt("wv0b", Wv0b, [2 * IN, 2 * H], f16)
            wv1b = wt("wv1b", Wv1b, [2 * H, 2 * H], f16)
            wv2b = wt("wv2b", Wv2b, [2 * H, 2], f16)
            b0 = wt("b0", b0v, [2 * H, 1], f32)
            b1 = wt("b1", b1v, [2 * H, 1], f32)

            def load_obs4(g):
                """obs4 [64, 2B]: rows 0:32 = ts {4g, 4g+1}, rows 32:64 =
                ts {4g+2, 4g+3} (two col blocks of B each)."""
                t0 = g * VG
                ob = obp.tile([2 * IN, B2], f16, name=f"obs4_{g}", tag="obs4")
                nc.sync.dma_start(
                    ob[0:IN, :].rearrange("k (g1 b) -> k g1 b", g1=2),
                    obsT16[t0:t0 + 2].transpose([1, 0, 2]))
                nc.sync.dma_start(
                    ob[IN:, :].rearrange("k (g1 b) -> k g1 b", g1=2),
                    obsT16[t0 + 2:t0 + 4].transpose([1, 0, 2]))
                return ob

            obs4 = load_obs4(0)
            obs4_next = load_obs4(1) if n_groups > 1 else None

            # xy_h [48, B] f16: rows 0:32 = y^T, rows 32:48 = x^T
            xt_r = xtp.tile([S, B], f32, name="xt_r0", tag="xt_r")
            nc.sync.dma_start(xt_r[:], x0T[:])
            xy_h = xyp.tile([S + IN, B], f16, name="xy_h0", tag="xy_h")
            nc.sync.dma_start(xy_h[0:IN, :], obsT16[0])
            nc.vector.tensor_copy(xy_h[IN:, :], xt_r[:])

            u4 = None
            u_pend = None  # (pu_tile, w_final, t): wuw matmul deferred to t+1
            u_last = None

            def copy_u(tp):
                nonlocal u4
                g4p = tp % VG
                if g4p == 0:
                    u4 = up.tile([OUT, VG * B], f32, name=f"u4_{tp // VG}", tag="u4")
                nc.vector.tensor_copy(u4[:, g4p * B:(g4p + 1) * B], u_last)
                if g4p == VG - 1:
                    nc.sync.dma_start(
                        u_out[tp - VG + 1:tp + 1].transpose([1, 0, 2]),
                        u4[:].rearrange("o (g1 b) -> o g1 b", g1=VG))

            for t in range(T):
                g, g4 = t // VG, t % VG
                # -- PE front: warm-start + it1-const (need only xy), value
                #    layer-0, then t-1's deferred u close.
                p0 = pwp.tile([NL, B], f32, name=f"pw{t}_0", tag="pw")
                nc.tensor.matmul(p0[:], wcd0[:], xy_h[:], start=True, stop=True)
                p1 = pwp.tile([NL, B], f32, name=f"pw{t}_1", tag="pw")
                nc.tensor.matmul(p1[:], wcd[:], xy_h[:], start=True, stop=False)
                if g4 == 0:
                    ph = php.tile([2 * H, B2], f32, name=f"ph_{g}", tag="ph")
                    nc.tensor.matmul(ph[:], wv0b[:], obs4[:], start=True, stop=True)
                w0 = wp.tile([NL, B], f16, name=f"w{t}_0", tag="w")
                nc.scalar.activation(w0[:], p0[:], AF.Tanh)           # tanh0
                if g4 == 0:
                    h1 = vp.tile([2 * H, B2], f16, name=f"h1_{g}", tag="h")
                    nc.scalar.activation(h1[:], ph[:], AF.Tanh, bias=b0[:])

                # -- x/u const halves fused into one matmul (rows 0:S = x,
                #    rows S:S+OUT = u), then the chain matmuls
                pxu = pxup.tile([32 + OUT, B], f32, name=f"pxu{t}", tag="pxu")
                nc.tensor.matmul(pxu[:], wxu[:], xy_h[:], start=True, stop=False)
                px = pxu[0:S, :]
                pu = pxu[32:, :]

                # w0-side terms first: x_next rows 0:S and u's -K*Duw rows
                # 32:40 gate the next timestep; w1 (tanh1) only feeds the
                # deferred u matmul, so its refold goes second on the PE.
                nc.tensor.matmul(pxu[:], wxuw[:], w0[:], start=False, stop=False,
                                 skip_group_check=True)
                nc.tensor.matmul(p1[:], wdvw[:], w0[:], start=False, stop=True)
                if u_pend is not None:
                    pu_p, w_p, _ = u_pend
                    nc.tensor.matmul(pu_p, wuw[:], w_p[:], start=False, stop=True,
                                     skip_group_check=True)
                    u_last, u_pend = pu_p, None
                if t < T - 1:
                    xy_hn = xyp.tile([S + IN, B], f16, name=f"xyh{t + 1}", tag="xy_h")
                    nc.sync.dma_start(xy_hn[0:IN, :], obsT16[t + 1])
                    nc.vector.tensor_add(xy_hn[IN:, :], px, xt_r[:])
                    if t < T - 2:
                        xt_rn = xtp.tile([S, B], f32, name=f"xtr{t + 1}", tag="xt_r")
                        nc.vector.tensor_add(xt_rn[:], px, xt_r[:])
                    else:
                        xt_rn = xt_r
                if u_last is not None and t > 0:
                    copy_u(t - 1)
                if g4 == 1:
                    ph2 = php.tile([2 * H, B2], f32, name=f"ph2_{g}", tag="ph")
                    nc.tensor.matmul(ph2[:], wv1b[:], h1[:], start=True, stop=True)

                # -- final refold iteration
                w1 = wp.tile([NL, B], f16, name=f"w{t}_1", tag="w")
                nc.scalar.activation(w1[:], p1[:], AF.Tanh)
                w_prev = w1
                if g4 == 1:
                    h2 = vp.tile([2 * H, B2], f16, name=f"h2_{g}", tag="h")
                    nc.scalar.activation(h2[:], ph2[:], AF.Tanh, bias=b1[:])

                # defer u's wuw matmul (needs final w) into t+1's PE queue
                u_pend = (pu, w_prev, t)

                if g4 == 2:
                    pv = pvp.tile([2, B2], f32, name=f"pv_{g}", tag="pv")
                    nc.tensor.matmul(pv[:], wv2b[:], h2[:], start=True, stop=True)
                    v_sb = vp.tile([2, B2], f32, name=f"v_sb{g}", tag="v_sb")
                    nc.vector.tensor_copy(v_sb[:], pv[:])
                    nc.sync.dma_start(
                        v_out[t - 2:t + 2].rearrange("(r g1) b -> r (g1 b)", r=2),
                        v_sb[:])
                if g4 == VG - 1:
                    # rotate prefetched obs group
                    obs4 = obs4_next
                    if g + 2 < n_groups:
                        obs4_next = load_obs4(g + 2)

                if t < T - 1:
                    xt_r, xy_h = xt_rn, xy_hn

            # close out the final timestep's u
            pu_p, w_p, _ = u_pend
            nc.tensor.matmul(pu_p, wuw[:], w_p[:], start=False, stop=True,
                             skip_group_check=True)
            u_last = pu_p
            copy_u(T - 1)

    nc.compile()
    return nc


def host_inputs(inputs, core, n_cores=N_CORES):
    BL = inputs["obs"].shape[0] // n_cores
    sl = slice(core * BL, (core + 1) * BL)
    obs = np.ascontiguousarray(np.asarray(inputs["obs"])[sl].transpose(1, 2, 0))
    x0T = np.ascontiguousarray(np.asarray(inputs["x0"])[sl].T)
    g = lambda k: np.asarray(inputs[k]).astype(np.float32)
    Dvw = g("Dvw_T")
    M = np.linalg.inv(np.eye(NL, dtype=np.float32) - G_INIT * Dvw)
    KI = np.linalg.inv(np.eye(NL, dtype=np.float32) - GK * Dvw)  # = I + K
    Wcd = np.concatenate([g("Dvy_T"), g("Cv_T")], 0)
    W0, W1, W2 = g("W0"), g("W1"), g("W2")
    Z = np.zeros_like
    blk = lambda A: np.block([[A, Z(A)], [Z(A), A]])
    return {
        "obsT16": obs.astype(np.float16),
        "x0T": x0T.astype(np.float32),
        "Wdvw": Dvw.astype(np.float16),
        "Wcd": Wcd.astype(np.float16),
        "Wcd0": (Wcd @ M).astype(np.float16),
        "Wxu": np.concatenate(
            [np.concatenate([DT * g("By_T"), DT * g("A_T")], 0),
             np.zeros((S + IN, 32 - S), np.float32),
             np.concatenate([g("Duy_T"), g("Cu_T")], 0)], 1).astype(np.float16),
        "Wuw": (KI @ g("Duw_T")).astype(np.float16),
        "Wxuw": np.concatenate(
            [DT * g("Bw_T"), np.zeros((NL, 32 - S), np.float32),
             -(KI - np.eye(NL, dtype=np.float32)) @ g("Duw_T")], 1
        ).astype(np.float16),
        "Wv0b": blk(W0).astype(np.float16),
        "Wv1b": blk(W1).astype(np.float16),
        "Wv2b": blk(W2).astype(np.float16),
        "b0v": np.tile(g("b0").reshape(H, 1), (2, 1)).astype(np.float32),
        "b1v": np.tile(g("b1").reshape(H, 1), (2, 1)).astype(np.float32),
    }


def assemble_output(results, inputs, n_cores=N_CORES):
    obs = np.asarray(inputs["obs"])
    Bfull, T = obs.shape[0], obs.shape[1]
    BL = Bfull // n_cores
    out = np.empty((Bfull, T, 2 * OUT + 1), np.float32)
    log_stds = np.asarray(inputs["log_stds"], np.float32)
    b2 = np.asarray(inputs["b2"], np.float32)
    for c in range(n_cores):
        sl = slice(c * BL, (c + 1) * BL)
        out[sl, :, :OUT] = results[c]["u_out"].transpose(2, 0, 1)
        out[sl, :, OUT:2 * OUT] = log_stds
        out[sl, :, 2 * OUT:] = results[c]["v_out"].T[:, :, None] + b2
    return out


_NC_CACHE = {}


def _get_nc(T):
    if T not in _NC_CACHE:
        _NC_CACHE[T] = build_kernel(T=T)
    return _NC_CACHE[T]


def run_on_hw(inputs, trace=False):
    """Run the SPMD kernel; returns (full_output, exec_time_ns_or_None)."""
    T = np.asarray(inputs["obs"]).shape[1]
    nc = _get_nc(T)
    in_maps = [host_inputs(inputs, c) for c in range(N_CORES)]
    last_err = None
    for attempt in range(3):
        try:
            res = run_bass_kernel_spmd(nc, in_maps, list(range(N_CORES)), trace=trace)
            return assemble_output(res.results, inputs), res.exec_time_ns
        except Exception as e:  # transient device failures: retry
            last_err = e
    raise last_err


def kernel(**inputs) -> np.ndarray:
    out, _ = run_on_hw(inputs, trace=False)
    return out


# revision 19
# speedup vs baseline: 1.5197x; 1.0949x over previous
"""DissipativeThetaRINN Trainium2 (Bass/Tile) kernel — 8-core data parallel.

Strategy (pure data parallel, per sharding hint):
  - Batch B=2048 is split across 8 NeuronCores (256 rows/core); the tiny
    controller matrices and value-MLP weights are replicated.
  - On-device layout is transposed: features on SBUF partitions, batch on
    the free dimension (256 columns per core).
  - The implicit layer w = tanh(Cv x + Dvy y + Dvw w) is solved with only
    N_ITERS=2 tanh evaluations: iteration 0 uses a linear-solve warm start
    w0 = tanh(M c) with M = (I - g Dvw)^-1, g=0.8, folded host-side into
    the const matmul (zero extra device work); one refold gives w1.  The
    u readout then applies a linear-solve correction for the remaining
    fixed-point residual: u += (w1 - w0) @ ((I - gK Dvw)^-1 - I) @ Duw,
    gK=0.75, folded host-side into the two u weight matrices (the w0-side
    term rides the x-update matmul).  All matmuls span the full 256-column
    batch — on TRN2 the per-instruction overhead (LDWEIGHTS ~100ns +
    drain ~170ns) dominates 128-col streams, so fewer, wider instructions
    beat a 2-chunk ping-pong.  Verified in a fp16-faithful numpy sim:
    rel_l2 = 4.6e-3 vs the 2e-2 budget.
  - x_next (forward Euler) uses the second-to-last w iterate so the final
    tanh is off the timestep-boundary critical path (error ~1e-5, verified
    in fp16-faithful numpy sim: total rel_l2 ≈ 5.5e-3 vs 2e-2 budget).
  - Matmuls run in fp16 (PSUM accumulates fp32); DT is pre-folded into the
    recurrence weights; the x accumulator stays fp32 on device.
  - The value MLP is computed in groups of 4 timesteps, packed 2-per-128
    partitions with block-diagonal weights, scheduled into engine gaps.
  - log_stds broadcast and the +b2 value bias are applied host-side.
"""
import numpy as np
import concourse.bass as bass
import concourse.mybir as mybir
import concourse.tile as tile
from concourse import bacc
from concourse.bass_utils import run_bass_kernel_spmd

dt = mybir.dt
AF = mybir.ActivationFunctionType

# problem shape (hardcoded per contract)
BFULL, TFULL = 2048, 128
S, NL, IN, OUT, H = 16, 128, 32, 8, 64
DT = 0.01
N_CORES = 8
N_ITERS = 2    # tanh evaluations per timestep (incl. warm-start iteration)
G_INIT = 0.8   # warm-start gain: w0 = tanh((I - g Dvw)^-T c)
GK = 0.75      # u-readout correction: u += (w1-w0) @ ((I-gK D)^-1 - I) @ Duw
VG = 4         # value-MLP timestep group (packed 2x2 onto 128 partitions)


def build_kernel(T=TFULL, B=BFULL // N_CORES, n_iters=N_ITERS):
    nc = bacc.Bacc(None, target_bir_lowering=False)
    f32, f16 = dt.float32, dt.float16
    assert n_iters >= 2

    obsT16 = nc.dram_tensor("obsT16", [T, IN, B], f16, kind="ExternalInput")
    x0T = nc.dram_tensor("x0T", [S, B], f32, kind="ExternalInput")
    Wdvw = nc.dram_tensor("Wdvw", [NL, NL], f16, kind="ExternalInput")
    Wcd = nc.dram_tensor("Wcd", [S + IN, NL], f16, kind="ExternalInput")
    Wcd0 = nc.dram_tensor("Wcd0", [S + IN, NL], f16, kind="ExternalInput")
    Wxu = nc.dram_tensor("Wxu", [S + IN, 32 + OUT], f16, kind="ExternalInput")
    Wuw = nc.dram_tensor("Wuw", [NL, OUT], f16, kind="ExternalInput")
    Wxuw = nc.dram_tensor("Wxuw", [NL, 32 + OUT], f16, kind="ExternalInput")
    Wv0b = nc.dram_tensor("Wv0b", [2 * IN, 2 * H], f16, kind="ExternalInput")
    Wv1b = nc.dram_tensor("Wv1b", [2 * H, 2 * H], f16, kind="ExternalInput")
    Wv2b = nc.dram_tensor("Wv2b", [2 * H, 2], f16, kind="ExternalInput")
    b0v = nc.dram_tensor("b0v", [2 * H, 1], f32, kind="ExternalInput")
    b1v = nc.dram_tensor("b1v", [2 * H, 1], f32, kind="ExternalInput")

    u_out = nc.dram_tensor("u_out", [T, OUT, B], f32, kind="ExternalOutput")
    v_out = nc.dram_tensor("v_out", [T, B], f32, kind="ExternalOutput")

    B2 = 2 * B
    n_groups = (T + VG - 1) // VG

    with tile.TileContext(nc) as tc:
        with tc.tile_pool(name="wts", bufs=1) as wts, \
             tc.tile_pool(name="xyp", bufs=3) as xyp, \
             tc.tile_pool(name="xtp", bufs=2) as xtp, \
             tc.tile_pool(name="wp", bufs=3) as wp, \
             tc.tile_pool(name="obp", bufs=2) as obp, \
             tc.tile_pool(name="vp", bufs=2) as vp, \
             tc.tile_pool(name="up", bufs=2) as up, \
             tc.tile_pool(name="pw", bufs=2, space="PSUM") as pwp, \
             tc.tile_pool(name="pxu", bufs=2, space="PSUM") as pxup, \
             tc.tile_pool(name="ph", bufs=1, space="PSUM") as php, \
             tc.tile_pool(name="pv", bufs=1, space="PSUM") as pvp:

            def wt(name, dram, shape, dtp):
                tl = wts.tile(shape, dtp, name=name)
                nc.sync.dma_start(tl[:], dram[:])
                return tl
            wdvw = wt("wdvw", Wdvw, [NL, NL], f16)
            wcd = wt("wcd", Wcd, [S + IN, NL], f16)
            wcd0 = wt("wcd0", Wcd0, [S + IN, NL], f16)
            wxu = wt("wxu", Wxu, [S + IN, 32 + OUT], f16)
            wuw = wt("wuw", Wuw, [NL, OUT], f16)
            wxuw = wt("wxuw", Wxuw, [NL, 32 + OUT], f16)
            wv0b = wt("wv0b", Wv0b, [2 * IN, 2 * H], f16)
            wv1b = wt("wv1b", Wv1b, [2 * H, 2 * H], f16)
            wv2b = wt("wv2b", Wv2b, [2 * H, 2], f16)
            b0 = wt("b0", b0v, [2 * H, 1], f32)
            b1 = wt("b1", b1v, [2 * H, 1], f32)

            def load_obs4(g):
                """obs4 [64, 2B]: rows 0:32 = ts {4g, 4g+1}, rows 32:64 =
                ts {4g+2, 4g+3} (two col blocks of B each)."""
                t0 = g * VG
                ob = obp.tile([2 * IN, B2], f16, name=f"obs4_{g}", tag="obs4")
                nc.sync.dma_start(
                    ob[0:IN, :].rearrange("k (g1 b) -> k g1 b", g1=2),
                    obsT16[t0:t0 + 2].transpose([1, 0, 2]))
                nc.sync.dma_start(
                    ob[IN:, :].rearrange("k (g1 b) -> k g1 b", g1=2),
                    obsT16[t0 + 2:t0 + 4].transpose([1, 0, 2]))
                return ob

            obs4 = load_obs4(0)
            obs4_next = load_obs4(1) if n_groups > 1 else None

            # xy_h [48, B] f16: rows 0:32 = y^T, rows 32:48 = x^T
            xt_r = xtp.tile([S, B], f32, name="xt_r0", tag="xt_r")
            nc.sync.dma_start(xt_r[:], x0T[:])
            xy_h = xyp.tile([S + IN, B], f16, name="xy_h0", tag="xy_h")
            nc.sync.dma_start(xy_h[0:IN, :], obsT16[0])
            nc.vector.tensor_copy(xy_h[IN:, :], xt_r[:])

            u4 = None
            u_pend = None  # (pu_tile, w_final, t): wuw matmul deferred to t+1
            u_last = None

            def copy_u(tp):
                nonlocal u4
                g4p = tp % VG
                if g4p == 0:
                    u4 = up.tile([OUT, VG * B], f32, name=f"u4_{tp // VG}", tag="u4")
                nc.vector.tensor_copy(u4[:, g4p * B:(g4p + 1) * B], u_last)
                if g4p == VG - 1:
                    nc.sync.dma_start(
                        u_out[tp - VG + 1:tp + 1].transpose([1, 0, 2]),
                        u4[:].rearrange("o (g1 b) -> o g1 b", g1=VG))

            for t in range(T):
                g, g4 = t // VG, t % VG
                # -- PE front: warm-start + it1-const (need only xy), value
                #    layer-0, then t-1's deferred u close.
                p0 = pwp.tile([NL, B], f32, name=f"pw{t}_0", tag="pw")
                nc.tensor.matmul(p0[:], wcd0[:], xy_h[:], start=True, stop=False,
                                 skip_group_check=True)
                if g4 == 0:
                    ph = php.tile([2 * H, B2], f32, name=f"ph_{g}", tag="ph")
                    nc.tensor.matmul(ph[:], wv0b[:], obs4[:], start=True, stop=True)
                w0 = wp.tile([NL, B], f16, name=f"w{t}_0", tag="w")
                nc.scalar.activation(w0[:], p0[:], AF.Tanh)           # tanh0
                if g4 == 0:
                    h1 = vp.tile([2 * H, B2], f16, name=f"h1_{g}", tag="h")
                    nc.scalar.activation(h1[:], ph[:], AF.Tanh, bias=b0[:])
                # implicit const: c = p0 (I - g Dvw), so the refold is
                # p1 = p0 + (w0 - g p0) Dvw — no second const matmul.
                sdl = wp.tile([NL, B], f16, name=f"s{t}", tag="s")
                nc.vector.scalar_tensor_tensor(
                    sdl[:], p0[:], -G_INIT, w0[:],
                    mybir.AluOpType.mult, mybir.AluOpType.add)

                # -- x/u const halves fused into one matmul (rows 0:S = x,
                #    rows S:S+OUT = u), then the chain matmuls
                pxu = pxup.tile([32 + OUT, B], f32, name=f"pxu{t}", tag="pxu")
                nc.tensor.matmul(pxu[:], wxu[:], xy_h[:], start=True, stop=False)
                px = pxu[0:S, :]
                pu = pxu[32:, :]

                # w0-side terms first: x_next rows 0:S and u's -K*Duw rows
                # 32:40 gate the next timestep; w1 (tanh1) only feeds the
                # deferred u matmul, so its refold goes second on the PE.
                nc.tensor.matmul(pxu[:], wxuw[:], w0[:], start=False, stop=False,
                                 skip_group_check=True)
                nc.tensor.matmul(p0[:], wdvw[:], sdl[:], start=False, stop=True,
                                 skip_group_check=True)
                if u_pend is not None:
                    pu_p, w_p, _ = u_pend
                    nc.tensor.matmul(pu_p, wuw[:], w_p[:], start=False, stop=True,
                                     skip_group_check=True)
                    u_last, u_pend = pu_p, None
                if t < T - 1:
                    xy_hn = xyp.tile([S + IN, B], f16, name=f"xyh{t + 1}", tag="xy_h")
                    nc.sync.dma_start(xy_hn[0:IN, :], obsT16[t + 1])
                    nc.vector.tensor_add(xy_hn[IN:, :], px, xt_r[:])
                    if t < T - 2:
                        xt_rn = xtp.tile([S, B], f32, name=f"xtr{t + 1}", tag="xt_r")
                        nc.vector.tensor_add(xt_rn[:], px, xt_r[:])
                    else:
                        xt_rn = xt_r
                if u_last is not None and t > 0:
                    copy_u(t - 1)
                if g4 == 1:
                    ph2 = php.tile([2 * H, B2], f32, name=f"ph2_{g}", tag="ph")
                    nc.tensor.matmul(ph2[:], wv1b[:], h1[:], start=True, stop=True)

                # -- final refold iteration
                w1 = wp.tile([NL, B], f16, name=f"w{t}_1", tag="w")
                nc.scalar.activation(w1[:], p0[:], AF.Tanh)
                w_prev = w1
                if g4 == 1:
                    h2 = vp.tile([2 * H, B2], f16, name=f"h2_{g}", tag="h")
                    nc.scalar.activation(h2[:], ph2[:], AF.Tanh, bias=b1[:])

                # defer u's wuw matmul (needs final w) into t+1's PE queue
                u_pend = (pu, w_prev, t)

                if g4 == 2:
                    pv = pvp.tile([2, B2], f32, name=f"pv_{g}", tag="pv")
                    nc.tensor.matmul(pv[:], wv2b[:], h2[:], start=True, stop=True)
                    v_sb = vp.tile([2, B2], f32, name=f"v_sb{g}", tag="v_sb")
                    nc.vector.tensor_copy(v_sb[:], pv[:])
                    nc.sync.dma_start(
                        v_out[t - 2:t + 2].rearrange("(r g1) b -> r (g1 b)", r=2),
                        v_sb[:])
                if g4 == VG - 1:
                    # rotate prefetched obs group
                    obs4 = obs4_next
                    if g + 2 < n_groups:
                        obs4_next = load_obs4(g + 2)

                if t < T - 1:
                    xt_r, xy_h = xt_rn, xy_hn

            # close out the final timestep's u
            pu_p, w_p, _ = u_pend
            nc.tensor.matmul(pu_p, wuw[:], w_p[:], start=False, stop=True,
                             skip_group_check=True)
            u_last = pu_p
            copy_u(T - 1)

    nc.compile()
    return nc


def host_inputs(inputs, core, n_cores=N_CORES):
    BL = inputs["obs"].shape[0] // n_cores
    sl = slice(core * BL, (core + 1) * BL)
    obs = np.ascontiguousarray(np.asarray(inputs["obs"])[sl].transpose(1, 2, 0))
    x0T = np.ascontiguousarray(np.asarray(inputs["x0"])[sl].T)
    g = lambda k: np.asarray(inputs[k]).astype(np.float32)
    Dvw = g("Dvw_T")
    M = np.linalg.inv(np.eye(NL, dtype=np.float32) - G_INIT * Dvw)
    KI = np.linalg.inv(np.eye(NL, dtype=np.float32) - GK * Dvw)  # = I + K
    Wcd = np.concatenate([g("Dvy_T"), g("Cv_T")], 0)
    W0, W1, W2 = g("W0"), g("W1"), g("W2")
    Z = np.zeros_like
    blk = lambda A: np.block([[A, Z(A)], [Z(A), A]])
    return {
        "obsT16": obs.astype(np.float16),
        "x0T": x0T.astype(np.float32),
        "Wdvw": Dvw.astype(np.float16),
        "Wcd": Wcd.astype(np.float16),
        "Wcd0": (Wcd @ M).astype(np.float16),
        "Wxu": np.concatenate(
            [np.concatenate([DT * g("By_T"), DT * g("A_T")], 0),
             np.zeros((S + IN, 32 - S), np.float32),
             np.concatenate([g("Duy_T"), g("Cu_T")], 0)], 1).astype(np.float16),
        "Wuw": (KI @ g("Duw_T")).astype(np.float16),
        "Wxuw": np.concatenate(
            [DT * g("Bw_T"), np.zeros((NL, 32 - S), np.float32),
             -(KI - np.eye(NL, dtype=np.float32)) @ g("Duw_T")], 1
        ).astype(np.float16),
        "Wv0b": blk(W0).astype(np.float16),
        "Wv1b": blk(W1).astype(np.float16),
        "Wv2b": blk(W2).astype(np.float16),
        "b0v": np.tile(g("b0").reshape(H, 1), (2, 1)).astype(np.float32),
        "b1v": np.tile(g("b1").reshape(H, 1), (2, 1)).astype(np.float32),
    }


def assemble_output(results, inputs, n_cores=N_CORES):
    obs = np.asarray(inputs["obs"])
    Bfull, T = obs.shape[0], obs.shape[1]
    BL = Bfull // n_cores
    out = np.empty((Bfull, T, 2 * OUT + 1), np.float32)
    log_stds = np.asarray(inputs["log_stds"], np.float32)
    b2 = np.asarray(inputs["b2"], np.float32)
    for c in range(n_cores):
        sl = slice(c * BL, (c + 1) * BL)
        out[sl, :, :OUT] = results[c]["u_out"].transpose(2, 0, 1)
        out[sl, :, OUT:2 * OUT] = log_stds
        out[sl, :, 2 * OUT:] = results[c]["v_out"].T[:, :, None] + b2
    return out


_NC_CACHE = {}


def _get_nc(T):
    if T not in _NC_CACHE:
        _NC_CACHE[T] = build_kernel(T=T)
    return _NC_CACHE[T]


def run_on_hw(inputs, trace=False):
    """Run the SPMD kernel; returns (full_output, exec_time_ns_or_None)."""
    T = np.asarray(inputs["obs"]).shape[1]
    nc = _get_nc(T)
    in_maps = [host_inputs(inputs, c) for c in range(N_CORES)]
    last_err = None
    for attempt in range(3):
        try:
            res = run_bass_kernel_spmd(nc, in_maps, list(range(N_CORES)), trace=trace)
            return assemble_output(res.results, inputs), res.exec_time_ns
        except Exception as e:  # transient device failures: retry
            last_err = e
    raise last_err


def kernel(**inputs) -> np.ndarray:
    out, _ = run_on_hw(inputs, trace=False)
    return out


# revision 20
# speedup vs baseline: 1.5230x; 1.0021x over previous
"""DissipativeThetaRINN Trainium2 (Bass/Tile) kernel — 8-core data parallel.

Strategy (pure data parallel, per sharding hint):
  - Batch B=2048 is split across 8 NeuronCores (256 rows/core); the tiny
    controller matrices and value-MLP weights are replicated.
  - On-device layout is transposed: features on SBUF partitions, batch on
    the free dimension (256 columns per core).
  - The implicit layer w = tanh(Cv x + Dvy y + Dvw w) is solved with only
    N_ITERS=2 tanh evaluations: iteration 0 uses a linear-solve warm start
    w0 = tanh(M c) with M = (I - g Dvw)^-1, g=0.8, folded host-side into
    the const matmul (zero extra device work); one refold gives w1.  The
    u readout then applies a linear-solve correction for the remaining
    fixed-point residual: u += (w1 - w0) @ ((I - gK Dvw)^-1 - I) @ Duw,
    gK=0.75, folded host-side into the two u weight matrices (the w0-side
    term rides the x-update matmul).  All matmuls span the full 256-column
    batch — on TRN2 the per-instruction overhead (LDWEIGHTS ~100ns +
    drain ~170ns) dominates 128-col streams, so fewer, wider instructions
    beat a 2-chunk ping-pong.  Verified in a fp16-faithful numpy sim:
    rel_l2 = 4.6e-3 vs the 2e-2 budget.
  - x_next (forward Euler) uses the second-to-last w iterate so the final
    tanh is off the timestep-boundary critical path (error ~1e-5, verified
    in fp16-faithful numpy sim: total rel_l2 ≈ 5.5e-3 vs 2e-2 budget).
  - Matmuls run in fp16 (PSUM accumulates fp32); DT is pre-folded into the
    recurrence weights; the x accumulator stays fp32 on device.
  - The value MLP is computed in groups of 4 timesteps, packed 2-per-128
    partitions with block-diagonal weights, scheduled into engine gaps.
  - log_stds broadcast and the +b2 value bias are applied host-side.
"""
import numpy as np
import concourse.bass as bass
import concourse.mybir as mybir
import concourse.tile as tile
from concourse import bacc
from concourse.bass_utils import run_bass_kernel_spmd

dt = mybir.dt
AF = mybir.ActivationFunctionType

# problem shape (hardcoded per contract)
BFULL, TFULL = 2048, 128
S, NL, IN, OUT, H = 16, 128, 32, 8, 64
DT = 0.01
N_CORES = 8
N_ITERS = 2    # tanh evaluations per timestep (incl. warm-start iteration)
G_INIT = 0.8   # warm-start gain: w0 = tanh((I - g Dvw)^-T c)
GK = 0.75      # u-readout correction: u += (w1-w0) @ ((I-gK D)^-1 - I) @ Duw
VG = 4         # value-MLP timestep group (packed 2x2 onto 128 partitions)


def build_kernel(T=TFULL, B=BFULL // N_CORES, n_iters=N_ITERS):
    nc = bacc.Bacc(None, target_bir_lowering=False)
    f32, f16 = dt.float32, dt.float16
    assert n_iters >= 2

    obsT16 = nc.dram_tensor("obsT16", [T, IN, B], f16, kind="ExternalInput")
    x0T = nc.dram_tensor("x0T", [S, B], f32, kind="ExternalInput")
    Wdvw = nc.dram_tensor("Wdvw", [NL, NL], f16, kind="ExternalInput")
    Wcd = nc.dram_tensor("Wcd", [S + IN, NL], f16, kind="ExternalInput")
    Wcd0 = nc.dram_tensor("Wcd0", [S + IN, NL], f16, kind="ExternalInput")
    Wxu = nc.dram_tensor("Wxu", [S + IN, 32 + OUT], f16, kind="ExternalInput")
    Wuw = nc.dram_tensor("Wuw", [NL, OUT], f16, kind="ExternalInput")
    Wxuw = nc.dram_tensor("Wxuw", [NL, 32 + OUT], f16, kind="ExternalInput")
    Wv0b = nc.dram_tensor("Wv0b", [2 * IN, 2 * H], f16, kind="ExternalInput")
    Wv1b = nc.dram_tensor("Wv1b", [2 * H, 2 * H], f16, kind="ExternalInput")
    Wv2b = nc.dram_tensor("Wv2b", [2 * H, 2], f16, kind="ExternalInput")
    b0v = nc.dram_tensor("b0v", [2 * H, 1], f32, kind="ExternalInput")
    b1v = nc.dram_tensor("b1v", [2 * H, 1], f32, kind="ExternalInput")

    u_out = nc.dram_tensor("u_out", [T, OUT, B], f32, kind="ExternalOutput")
    v_out = nc.dram_tensor("v_out", [T, B], f32, kind="ExternalOutput")

    B2 = 2 * B
    n_groups = (T + VG - 1) // VG

    with tile.TileContext(nc) as tc:
        with tc.tile_pool(name="wts", bufs=1) as wts, \
             tc.tile_pool(name="xyp", bufs=3) as xyp, \
             tc.tile_pool(name="xtp", bufs=2) as xtp, \
             tc.tile_pool(name="wp", bufs=3) as wp, \
             tc.tile_pool(name="obp", bufs=2) as obp, \
             tc.tile_pool(name="vp", bufs=2) as vp, \
             tc.tile_pool(name="up", bufs=2) as up, \
             tc.tile_pool(name="pw", bufs=2, space="PSUM") as pwp, \
             tc.tile_pool(name="pxu", bufs=2, space="PSUM") as pxup, \
             tc.tile_pool(name="ph", bufs=1, space="PSUM") as php, \
             tc.tile_pool(name="pv", bufs=1, space="PSUM") as pvp:

            def wt(name, dram, shape, dtp):
                tl = wts.tile(shape, dtp, name=name)
                nc.sync.dma_start(tl[:], dram[:])
                return tl
            wdvw = wt("wdvw", Wdvw, [NL, NL], f16)
            wcd = wt("wcd", Wcd, [S + IN, NL], f16)
            wcd0 = wt("wcd0", Wcd0, [S + IN, NL], f16)
            wxu = wt("wxu", Wxu, [S + IN, 32 + OUT], f16)
            wuw = wt("wuw", Wuw, [NL, OUT], f16)
            wxuw = wt("wxuw", Wxuw, [NL, 32 + OUT], f16)
            wv0b = wt("wv0b", Wv0b, [2 * IN, 2 * H], f16)
            wv1b = wt("wv1b", Wv1b, [2 * H, 2 * H], f16)
            wv2b = wt("wv2b", Wv2b, [2 * H, 2], f16)
            b0 = wt("b0", b0v, [2 * H, 1], f32)
            b1 = wt("b1", b1v, [2 * H, 1], f32)

            def load_obs4(g):
                """obs4 [64, 2B]: rows 0:32 = ts {4g, 4g+1}, rows 32:64 =
                ts {4g+2, 4g+3} (two col blocks of B each)."""
                t0 = g * VG
                ob = obp.tile([2 * IN, B2], f16, name=f"obs4_{g}", tag="obs4")
                nc.sync.dma_start(
                    ob[0:IN, :].rearrange("k (g1 b) -> k g1 b", g1=2),
                    obsT16[t0:t0 + 2].transpose([1, 0, 2]))
                nc.sync.dma_start(
                    ob[IN:, :].rearrange("k (g1 b) -> k g1 b", g1=2),
                    obsT16[t0 + 2:t0 + 4].transpose([1, 0, 2]))
                return ob

            obs4 = load_obs4(0)

            # xy_h [48, B] f16: rows 0:32 = y^T, rows 32:48 = x^T
            xt_r = xtp.tile([S, B], f32, name="xt_r0", tag="xt_r")
            nc.sync.dma_start(xt_r[:], x0T[:])
            xy_h = xyp.tile([S + IN, B], f16, name="xy_h0", tag="xy_h")
            nc.sync.dma_start(xy_h[0:IN, :], obsT16[0])
            nc.vector.tensor_copy(xy_h[IN:, :], xt_r[:])

            u4 = None
            u_pend = None  # (pu_tile, w_final, t): wuw matmul deferred to t+1
            u_last = None

            def copy_u(tp):
                nonlocal u4
                g4p = tp % VG
                if g4p == 0:
                    u4 = up.tile([OUT, VG * B], f32, name=f"u4_{tp // VG}", tag="u4")
                nc.vector.tensor_copy(u4[:, g4p * B:(g4p + 1) * B], u_last)
                if g4p == VG - 1:
                    nc.sync.dma_start(
                        u_out[tp - VG + 1:tp + 1].transpose([1, 0, 2]),
                        u4[:].rearrange("o (g1 b) -> o g1 b", g1=VG))

            for t in range(T):
                g, g4 = t // VG, t % VG
                # -- PE front: warm-start + it1-const (need only xy), value
                #    layer-0, then t-1's deferred u close.
                p0 = pwp.tile([NL, B], f32, name=f"pw{t}_0", tag="pw")
                nc.tensor.matmul(p0[:], wcd0[:], xy_h[:], start=True, stop=False,
                                 skip_group_check=True)
                if g4 == 0:
                    ph = php.tile([2 * H, B2], f32, name=f"ph_{g}", tag="ph")
                    nc.tensor.matmul(ph[:], wv0b[:], obs4[:], start=True, stop=True)
                w0 = wp.tile([NL, B], f16, name=f"w{t}_0", tag="w")
                nc.scalar.activation(w0[:], p0[:], AF.Tanh)           # tanh0
                if g4 == 0:
                    h1 = vp.tile([2 * H, B2], f16, name=f"h1_{g}", tag="h")
                    nc.scalar.activation(h1[:], ph[:], AF.Tanh, bias=b0[:])
                # implicit const: c = p0 (I - g Dvw), so the refold is
                # p1 = p0 + (w0 - g p0) Dvw — no second const matmul.
                sdl = wp.tile([NL, B], f16, name=f"s{t}", tag="s")
                nc.vector.scalar_tensor_tensor(
                    sdl[:], p0[:], -G_INIT, w0[:],
                    mybir.AluOpType.mult, mybir.AluOpType.add)

                # -- x/u const halves fused into one matmul (rows 0:S = x,
                #    rows S:S+OUT = u), then the chain matmuls
                pxu = pxup.tile([32 + OUT, B], f32, name=f"pxu{t}", tag="pxu")
                nc.tensor.matmul(pxu[:], wxu[:], xy_h[:], start=True, stop=False)
                px = pxu[0:S, :]
                pu = pxu[32:, :]

                # w0-side terms first: x_next rows 0:S and u's -K*Duw rows
                # 32:40 gate the next timestep; w1 (tanh1) only feeds the
                # deferred u matmul, so its refold goes second on the PE.
                nc.tensor.matmul(pxu[:], wxuw[:], w0[:], start=False, stop=False,
                                 skip_group_check=True)
                nc.tensor.matmul(p0[:], wdvw[:], sdl[:], start=False, stop=True,
                                 skip_group_check=True)
                if u_pend is not None:
                    pu_p, w_p, _ = u_pend
                    nc.tensor.matmul(pu_p, wuw[:], w_p[:], start=False, stop=True,
                                     skip_group_check=True)
                    u_last, u_pend = pu_p, None
                if t < T - 1:
                    xy_hn = xyp.tile([S + IN, B], f16, name=f"xyh{t + 1}", tag="xy_h")
                    nc.sync.dma_start(xy_hn[0:IN, :], obsT16[t + 1])
                    nc.vector.tensor_add(xy_hn[IN:, :], px, xt_r[:])
                    if t < T - 2:
                        xt_rn = xtp.tile([S, B], f32, name=f"xtr{t + 1}", tag="xt_r")
                        nc.vector.tensor_add(xt_rn[:], px, xt_r[:])
                    else:
                        xt_rn = xt_r
                if u_last is not None and t > 0:
                    copy_u(t - 1)
                if g4 == 1:
                    ph2 = php.tile([2 * H, B2], f32, name=f"ph2_{g}", tag="ph")
                    nc.tensor.matmul(ph2[:], wv1b[:], h1[:], start=True, stop=True)

                # -- final refold iteration
                w1 = wp.tile([NL, B], f16, name=f"w{t}_1", tag="w")
                nc.scalar.activation(w1[:], p0[:], AF.Tanh)
                w_prev = w1
                if g4 == 1:
                    h2 = vp.tile([2 * H, B2], f16, name=f"h2_{g}", tag="h")
                    nc.scalar.activation(h2[:], ph2[:], AF.Tanh, bias=b1[:])

                # defer u's wuw matmul (needs final w) into t+1's PE queue
                u_pend = (pu, w_prev, t)

                if g4 == 2:
                    pv = pvp.tile([2, B2], f32, name=f"pv_{g}", tag="pv")
                    nc.tensor.matmul(pv[:], wv2b[:], h2[:], start=True, stop=True)
                    v_sb = vp.tile([2, B2], f32, name=f"v_sb{g}", tag="v_sb")
                    nc.vector.tensor_copy(v_sb[:], pv[:])
                    nc.sync.dma_start(
                        v_out[t - 2:t + 2].rearrange("(r g1) b -> r (g1 b)", r=2),
                        v_sb[:])
                if g4 == VG - 1:
                    # load the NEXT group's obs here (1 group of lookahead,
                    # paced by the in-order SP queue): throttles the value
                    # MLP so its PE work spreads into the recurrence's idle
                    # slots instead of congesting the first ~30 timesteps
                    if g + 1 < n_groups:
                        obs4 = load_obs4(g + 1)

                if t < T - 1:
                    xt_r, xy_h = xt_rn, xy_hn

            # close out the final timestep's u
            pu_p, w_p, _ = u_pend
            nc.tensor.matmul(pu_p, wuw[:], w_p[:], start=False, stop=True,
                             skip_group_check=True)
            u_last = pu_p
            copy_u(T - 1)

    nc.compile()
    return nc


def host_inputs(inputs, core, n_cores=N_CORES):
    BL = inputs["obs"].shape[0] // n_cores
    sl = slice(core * BL, (core + 1) * BL)
    obs = np.ascontiguousarray(np.asarray(inputs["obs"])[sl].transpose(1, 2, 0))
    x0T = np.ascontiguousarray(np.asarray(inputs["x0"])[sl].T)
    g = lambda k: np.asarray(inputs[k]).astype(np.float32)
    Dvw = g("Dvw_T")
    M = np.linalg.inv(np.eye(NL, dtype=np.float32) - G_INIT * Dvw)
    KI = np.linalg.inv(np.eye(NL, dtype=np.float32) - GK * Dvw)  # = I + K
    Wcd = np.concatenate([g("Dvy_T"), g("Cv_T")], 0)
    W0, W1, W2 = g("W0"), g("W1"), g("W2")
    Z = np.zeros_like
    blk = lambda A: np.block([[A, Z(A)], [Z(A), A]])
    return {
        "obsT16": obs.astype(np.float16),
        "x0T": x0T.astype(np.float32),
        "Wdvw": Dvw.astype(np.float16),
        "Wcd": Wcd.astype(np.float16),
        "Wcd0": (Wcd @ M).astype(np.float16),
        "Wxu": np.concatenate(
            [np.concatenate([DT * g("By_T"), DT * g("A_T")], 0),
             np.zeros((S + IN, 32 - S), np.float32),
             np.concatenate([g("Duy_T"), g("Cu_T")], 0)], 1).astype(np.float16),
        "Wuw": (KI @ g("Duw_T")).astype(np.float16),
        "Wxuw": np.concatenate(
            [DT * g("Bw_T"), np.zeros((NL, 32 - S), np.float32),
             -(KI - np.eye(NL, dtype=np.float32)) @ g("Duw_T")], 1
        ).astype(np.float16),
        "Wv0b": blk(W0).astype(np.float16),
        "Wv1b": blk(W1).astype(np.float16),
        "Wv2b": blk(W2).astype(np.float16),
        "b0v": np.tile(g("b0").reshape(H, 1), (2, 1)).astype(np.float32),
        "b1v": np.tile(g("b1").reshape(H, 1), (2, 1)).astype(np.float32),
    }


def assemble_output(results, inputs, n_cores=N_CORES):
    obs = np.asarray(inputs["obs"])
    Bfull, T = obs.shape[0], obs.shape[1]
    BL = Bfull // n_cores
    out = np.empty((Bfull, T, 2 * OUT + 1), np.float32)
    log_stds = np.asarray(inputs["log_stds"], np.float32)
    b2 = np.asarray(inputs["b2"], np.float32)
    for c in range(n_cores):
        sl = slice(c * BL, (c + 1) * BL)
        out[sl, :, :OUT] = results[c]["u_out"].transpose(2, 0, 1)
        out[sl, :, OUT:2 * OUT] = log_stds
        out[sl, :, 2 * OUT:] = results[c]["v_out"].T[:, :, None] + b2
    return out


_NC_CACHE = {}


def _get_nc(T):
    if T not in _NC_CACHE:
        _NC_CACHE[T] = build_kernel(T=T)
    return _NC_CACHE[T]


def run_on_hw(inputs, trace=False):
    """Run the SPMD kernel; returns (full_output, exec_time_ns_or_None)."""
    T = np.asarray(inputs["obs"]).shape[1]
    nc = _get_nc(T)
    in_maps = [host_inputs(inputs, c) for c in range(N_CORES)]
    last_err = None
    for attempt in range(3):
        try:
            res = run_bass_kernel_spmd(nc, in_maps, list(range(N_CORES)), trace=trace)
            return assemble_output(res.results, inputs), res.exec_time_ns
        except Exception as e:  # transient device failures: retry
            last_err = e
    raise last_err


def kernel(**inputs) -> np.ndarray:
    out, _ = run_on_hw(inputs, trace=False)
    return out


# revision 21
# speedup vs baseline: 1.5353x; 1.0081x over previous
"""DissipativeThetaRINN Trainium2 (Bass/Tile) kernel — 8-core data parallel.

Strategy (pure data parallel, per sharding hint):
  - Batch B=2048 is split across 8 NeuronCores (256 rows/core); the tiny
    controller matrices and value-MLP weights are replicated.
  - On-device layout is transposed: features on SBUF partitions, batch on
    the free dimension (256 columns per core).
  - The implicit layer w = tanh(Cv x + Dvy y + Dvw w) is solved with only
    N_ITERS=2 tanh evaluations: iteration 0 uses a linear-solve warm start
    w0 = tanh(M c) with M = (I - g Dvw)^-1, g=0.8, folded host-side into
    the const matmul (zero extra device work); one refold gives w1.  The
    u readout then applies a linear-solve correction for the remaining
    fixed-point residual: u += (w1 - w0) @ ((I - gK Dvw)^-1 - I) @ Duw,
    gK=0.75, folded host-side into the two u weight matrices (the w0-side
    term rides the x-update matmul).  All matmuls span the full 256-column
    batch — on TRN2 the per-instruction overhead (LDWEIGHTS ~100ns +
    drain ~170ns) dominates 128-col streams, so fewer, wider instructions
    beat a 2-chunk ping-pong.  Verified in a fp16-faithful numpy sim:
    rel_l2 = 4.6e-3 vs the 2e-2 budget.
  - x_next (forward Euler) uses the second-to-last w iterate so the final
    tanh is off the timestep-boundary critical path (error ~1e-5, verified
    in fp16-faithful numpy sim: total rel_l2 ≈ 5.5e-3 vs 2e-2 budget).
  - Matmuls run in fp16 (PSUM accumulates fp32); DT is pre-folded into the
    recurrence weights; the x accumulator stays fp32 on device.
  - The value MLP is computed in groups of 4 timesteps, packed 2-per-128
    partitions with block-diagonal weights, scheduled into engine gaps.
  - log_stds broadcast and the +b2 value bias are applied host-side.
"""
import numpy as np
import concourse.bass as bass
import concourse.mybir as mybir
import concourse.tile as tile
from concourse import bacc
from concourse.bass_utils import run_bass_kernel_spmd

dt = mybir.dt
AF = mybir.ActivationFunctionType

# problem shape (hardcoded per contract)
BFULL, TFULL = 2048, 128
S, NL, IN, OUT, H = 16, 128, 32, 8, 64
DT = 0.01
N_CORES = 8
N_ITERS = 2    # tanh evaluations per timestep (incl. warm-start iteration)
G_INIT = 0.8   # warm-start gain: w0 = tanh((I - g Dvw)^-T c)
GK = 0.75      # u-readout correction: u += (w1-w0) @ ((I-gK D)^-1 - I) @ Duw
VG = 4         # value-MLP timestep group (packed 2x2 onto 128 partitions)


def build_kernel(T=TFULL, B=BFULL // N_CORES, n_iters=N_ITERS):
    nc = bacc.Bacc(None, target_bir_lowering=False)
    f32, f16 = dt.float32, dt.float16
    assert n_iters >= 2

    obsT16 = nc.dram_tensor("obsT16", [T, IN, B], f16, kind="ExternalInput")
    x0T = nc.dram_tensor("x0T", [S, B], f32, kind="ExternalInput")
    Wdvw = nc.dram_tensor("Wdvw", [NL, NL], f16, kind="ExternalInput")
    Wcd = nc.dram_tensor("Wcd", [S + IN, NL], f16, kind="ExternalInput")
    Wcd0 = nc.dram_tensor("Wcd0", [S + IN, NL], f16, kind="ExternalInput")
    Wxu = nc.dram_tensor("Wxu", [S + IN, 32 + OUT], f16, kind="ExternalInput")
    Wuw = nc.dram_tensor("Wuw", [NL, OUT], f16, kind="ExternalInput")
    Wxuw = nc.dram_tensor("Wxuw", [NL, 32 + OUT], f16, kind="ExternalInput")
    Wv0b = nc.dram_tensor("Wv0b", [2 * IN, 2 * H], f16, kind="ExternalInput")
    Wv1b = nc.dram_tensor("Wv1b", [2 * H, 2 * H], f16, kind="ExternalInput")
    Wv2b = nc.dram_tensor("Wv2b", [2 * H, 2], f16, kind="ExternalInput")
    b0v = nc.dram_tensor("b0v", [2 * H, 1], f32, kind="ExternalInput")
    b1v = nc.dram_tensor("b1v", [2 * H, 1], f32, kind="ExternalInput")

    u_out = nc.dram_tensor("u_out", [T, OUT, B], f32, kind="ExternalOutput")
    v_out = nc.dram_tensor("v_out", [T, B], f32, kind="ExternalOutput")

    B2 = 2 * B
    n_groups = (T + VG - 1) // VG

    with tile.TileContext(nc) as tc:
        with tc.tile_pool(name="wts", bufs=1) as wts, \
             tc.tile_pool(name="xyp", bufs=3) as xyp, \
             tc.tile_pool(name="xtp", bufs=2) as xtp, \
             tc.tile_pool(name="wp", bufs=3) as wp, \
             tc.tile_pool(name="obp", bufs=2) as obp, \
             tc.tile_pool(name="vp", bufs=2) as vp, \
             tc.tile_pool(name="up", bufs=2) as up, \
             tc.tile_pool(name="pw", bufs=2, space="PSUM") as pwp, \
             tc.tile_pool(name="pxu", bufs=2, space="PSUM") as pxup, \
             tc.tile_pool(name="ph", bufs=1, space="PSUM") as php, \
             tc.tile_pool(name="pv", bufs=1, space="PSUM") as pvp:

            def wt(name, dram, shape, dtp):
                tl = wts.tile(shape, dtp, name=name)
                nc.sync.dma_start(tl[:], dram[:])
                return tl
            # first-timestep data before the weight loads: DMA configs
            # serialize on the Sync queue (~640ns each), so the order here
            # sets how soon the first it0 matmul can issue.
            xt_r = xtp.tile([S, B], f32, name="xt_r0", tag="xt_r")
            nc.sync.dma_start(xt_r[:], x0T[:])
            xy_h = xyp.tile([S + IN, B], f16, name="xy_h0", tag="xy_h")
            nc.sync.dma_start(xy_h[0:IN, :], obsT16[0])
            wcd0 = wt("wcd0", Wcd0, [S + IN, NL], f16)
            wdvw = wt("wdvw", Wdvw, [NL, NL], f16)
            wxu = wt("wxu", Wxu, [S + IN, 32 + OUT], f16)
            wxuw = wt("wxuw", Wxuw, [NL, OUT + 32], f16)
            wuw = wt("wuw", Wuw, [NL, OUT], f16)
            nc.vector.tensor_copy(xy_h[IN:, :], xt_r[:])
            wv0b = wt("wv0b", Wv0b, [2 * IN, 2 * H], f16)
            wv1b = wt("wv1b", Wv1b, [2 * H, 2 * H], f16)
            wv2b = wt("wv2b", Wv2b, [2 * H, 2], f16)
            b0 = wt("b0", b0v, [2 * H, 1], f32)
            b1 = wt("b1", b1v, [2 * H, 1], f32)

            def load_obs4(g):
                """obs4 [64, 2B]: rows 0:32 = ts {4g, 4g+1}, rows 32:64 =
                ts {4g+2, 4g+3} (two col blocks of B each)."""
                t0 = g * VG
                ob = obp.tile([2 * IN, B2], f16, name=f"obs4_{g}", tag="obs4")
                nc.sync.dma_start(
                    ob[0:IN, :].rearrange("k (g1 b) -> k g1 b", g1=2),
                    obsT16[t0:t0 + 2].transpose([1, 0, 2]))
                nc.sync.dma_start(
                    ob[IN:, :].rearrange("k (g1 b) -> k g1 b", g1=2),
                    obsT16[t0 + 2:t0 + 4].transpose([1, 0, 2]))
                return ob

            obs4 = load_obs4(0)

            u4 = None
            u_pend = None  # (pu_tile, w_final, t): wuw matmul deferred to t+1
            u_last = None

            def copy_u(tp):
                nonlocal u4
                g4p = tp % VG
                if g4p == 0:
                    u4 = up.tile([OUT, VG * B], f32, name=f"u4_{tp // VG}", tag="u4")
                nc.vector.tensor_copy(u4[:, g4p * B:(g4p + 1) * B], u_last)
                if g4p == VG - 1:
                    nc.sync.dma_start(
                        u_out[tp - VG + 1:tp + 1].transpose([1, 0, 2]),
                        u4[:].rearrange("o (g1 b) -> o g1 b", g1=VG))

            for t in range(T):
                g, g4 = t // VG, t % VG
                # -- PE front: warm-start + it1-const (need only xy), value
                #    layer-0, then t-1's deferred u close.
                p0 = pwp.tile([NL, B], f32, name=f"pw{t}_0", tag="pw")
                nc.tensor.matmul(p0[:], wcd0[:], xy_h[:], start=True, stop=False,
                                 skip_group_check=True)
                if g4 == 0:
                    ph = php.tile([2 * H, B2], f32, name=f"ph_{g}", tag="ph")
                    nc.tensor.matmul(ph[:], wv0b[:], obs4[:], start=True, stop=True)
                w0 = wp.tile([NL, B], f16, name=f"w{t}_0", tag="w")
                nc.scalar.activation(w0[:], p0[:], AF.Tanh)           # tanh0
                if g4 == 0:
                    h1 = vp.tile([2 * H, B2], f16, name=f"h1_{g}", tag="h")
                    nc.scalar.activation(h1[:], ph[:], AF.Tanh, bias=b0[:])
                # implicit const: c = p0 (I - g Dvw), so the refold is
                # p1 = p0 + (w0 - g p0) Dvw — no second const matmul.
                sdl = wp.tile([NL, B], f16, name=f"s{t}", tag="s")
                nc.vector.scalar_tensor_tensor(
                    sdl[:], p0[:], -G_INIT, w0[:],
                    mybir.AluOpType.mult, mybir.AluOpType.add)

                # -- x/u const halves fused into one matmul (rows 0:S = x,
                #    rows S:S+OUT = u), then the chain matmuls
                pxu = pxup.tile([32 + OUT, B], f32, name=f"pxu{t}", tag="pxu")
                nc.tensor.matmul(pxu[:], wxu[:], xy_h[:], start=True, stop=False)
                px = pxu[0:S, :]
                pu = pxu[32:, :]

                # w0-side terms first: x_next rows 0:S and u's -K*Duw rows
                # 32:40 gate the next timestep; w1 (tanh1) only feeds the
                # deferred u matmul, so its refold goes second on the PE.
                nc.tensor.matmul(pxu[:], wxuw[:], w0[:], start=False, stop=False,
                                 skip_group_check=True)
                nc.tensor.matmul(p0[:], wdvw[:], sdl[:], start=False, stop=True,
                                 skip_group_check=True)
                if u_pend is not None:
                    pu_p, w_p, _ = u_pend
                    nc.tensor.matmul(pu_p, wuw[:], w_p[:], start=False, stop=True,
                                     skip_group_check=True)
                    u_last, u_pend = pu_p, None
                if t < T - 1:
                    xy_hn = xyp.tile([S + IN, B], f16, name=f"xyh{t + 1}", tag="xy_h")
                    nc.sync.dma_start(xy_hn[0:IN, :], obsT16[t + 1])
                    nc.vector.tensor_add(xy_hn[IN:, :], px, xt_r[:])
                    if t < T - 2:
                        xt_rn = xtp.tile([S, B], f32, name=f"xtr{t + 1}", tag="xt_r")
                        nc.vector.tensor_add(xt_rn[:], px, xt_r[:])
                    else:
                        xt_rn = xt_r
                if u_last is not None and t > 0:
                    copy_u(t - 1)
                if g4 == 1:
                    ph2 = php.tile([2 * H, B2], f32, name=f"ph2_{g}", tag="ph")
                    nc.tensor.matmul(ph2[:], wv1b[:], h1[:], start=True, stop=True)

                # -- final refold iteration
                w1 = wp.tile([NL, B], f16, name=f"w{t}_1", tag="w")
                nc.scalar.activation(w1[:], p0[:], AF.Tanh)
                w_prev = w1
                if g4 == 1:
                    h2 = vp.tile([2 * H, B2], f16, name=f"h2_{g}", tag="h")
                    nc.scalar.activation(h2[:], ph2[:], AF.Tanh, bias=b1[:])

                # defer u's wuw matmul (needs final w) into t+1's PE queue
                u_pend = (pu, w_prev, t)

                if g4 == 2:
                    pv = pvp.tile([2, B2], f32, name=f"pv_{g}", tag="pv")
                    nc.tensor.matmul(pv[:], wv2b[:], h2[:], start=True, stop=True)
                    v_sb = vp.tile([2, B2], f32, name=f"v_sb{g}", tag="v_sb")
                    nc.vector.tensor_copy(v_sb[:], pv[:])
                    nc.sync.dma_start(
                        v_out[t - 2:t + 2].rearrange("(r g1) b -> r (g1 b)", r=2),
                        v_sb[:])
                if g4 == VG - 1:
                    # load the NEXT group's obs here (1 group of lookahead,
                    # paced by the in-order SP queue): throttles the value
                    # MLP so its PE work spreads into the recurrence's idle
                    # slots instead of congesting the first ~30 timesteps
                    if g + 1 < n_groups:
                        obs4 = load_obs4(g + 1)

                if t < T - 1:
                    xt_r, xy_h = xt_rn, xy_hn

            # close out the final timestep's u
            pu_p, w_p, _ = u_pend
            nc.tensor.matmul(pu_p, wuw[:], w_p[:], start=False, stop=True,
                             skip_group_check=True)
            u_last = pu_p
            copy_u(T - 1)

    nc.compile()
    return nc


def host_inputs(inputs, core, n_cores=N_CORES):
    BL = inputs["obs"].shape[0] // n_cores
    sl = slice(core * BL, (core + 1) * BL)
    obs = np.ascontiguousarray(np.asarray(inputs["obs"])[sl].transpose(1, 2, 0))
    x0T = np.ascontiguousarray(np.asarray(inputs["x0"])[sl].T)
    g = lambda k: np.asarray(inputs[k]).astype(np.float32)
    Dvw = g("Dvw_T")
    M = np.linalg.inv(np.eye(NL, dtype=np.float32) - G_INIT * Dvw)
    KI = np.linalg.inv(np.eye(NL, dtype=np.float32) - GK * Dvw)  # = I + K
    Wcd = np.concatenate([g("Dvy_T"), g("Cv_T")], 0)
    W0, W1, W2 = g("W0"), g("W1"), g("W2")
    Z = np.zeros_like
    blk = lambda A: np.block([[A, Z(A)], [Z(A), A]])
    return {
        "obsT16": obs.astype(np.float16),
        "x0T": x0T.astype(np.float32),
        "Wdvw": Dvw.astype(np.float16),
        "Wcd": Wcd.astype(np.float16),
        "Wcd0": (Wcd @ M).astype(np.float16),
        "Wxu": np.concatenate(
            [np.concatenate([DT * g("By_T"), DT * g("A_T")], 0),
             np.zeros((S + IN, 32 - S), np.float32),
             np.concatenate([g("Duy_T"), g("Cu_T")], 0)], 1).astype(np.float16),
        "Wuw": (KI @ g("Duw_T")).astype(np.float16),
        "Wxuw": np.concatenate(
            [DT * g("Bw_T"), np.zeros((NL, 32 - S), np.float32),
             -(KI - np.eye(NL, dtype=np.float32)) @ g("Duw_T")], 1
        ).astype(np.float16),
        "Wv0b": blk(W0).astype(np.float16),
        "Wv1b": blk(W1).astype(np.float16),
        "Wv2b": blk(W2).astype(np.float16),
        "b0v": np.tile(g("b0").reshape(H, 1), (2, 1)).astype(np.float32),
        "b1v": np.tile(g("b1").reshape(H, 1), (2, 1)).astype(np.float32),
    }


def assemble_output(results, inputs, n_cores=N_CORES):
    obs = np.asarray(inputs["obs"])
    Bfull, T = obs.shape[0], obs.shape[1]
    BL = Bfull // n_cores
    out = np.empty((Bfull, T, 2 * OUT + 1), np.float32)
    log_stds = np.asarray(inputs["log_stds"], np.float32)
    b2 = np.asarray(inputs["b2"], np.float32)
    for c in range(n_cores):
        sl = slice(c * BL, (c + 1) * BL)
        out[sl, :, :OUT] = results[c]["u_out"].transpose(2, 0, 1)
        out[sl, :, OUT:2 * OUT] = log_stds
        out[sl, :, 2 * OUT:] = results[c]["v_out"].T[:, :, None] + b2
    return out


_NC_CACHE = {}


def _get_nc(T):
    if T not in _NC_CACHE:
        _NC_CACHE[T] = build_kernel(T=T)
    return _NC_CACHE[T]


def run_on_hw(inputs, trace=False):
    """Run the SPMD kernel; returns (full_output, exec_time_ns_or_None)."""
    T = np.asarray(inputs["obs"]).shape[1]
    nc = _get_nc(T)
    in_maps = [host_inputs(inputs, c) for c in range(N_CORES)]
    last_err = None
    for attempt in range(3):
        try:
            res = run_bass_kernel_spmd(nc, in_maps, list(range(N_CORES)), trace=trace)
            return assemble_output(res.results, inputs), res.exec_time_ns
        except Exception as e:  # transient device failures: retry
            last_err = e
    raise last_err


def kernel(**inputs) -> np.ndarray:
    out, _ = run_on_hw(inputs, trace=False)
    return out
